# revision 35
# baseline (speedup 1.0000x reference)
"""Trainium2 Bass kernel for nn_DQNSolverCNN6 (Sudoku DQN with ACT halting).

Self-contained: host-side weight folding + 8-core SPMD bass kernel.

Structure (per core, batch shard BS=256, tokens T=BS*81):
  prologue: conv branches folded into 3 (128x90) matmuls + BN + ReLU -> s0
  2 transformer steps (global halting freezes the net after step 2 for any
  realistic input; host validates via halt-max outputs and falls back to a
  numpy path if the input behaves differently)
  epilogue: q = Wfc @ (sp1*ns1 + (1-sp1)*ns2) + bias, ponder from hp1/hp2.

Layouts: activations (128 d-partitions, T tokens) fp32 in SBUF; attention
per-element with (j, q) score layout; LayerNorm via PE-transpose land.
"""
import math
import numpy as np

B_FULL, D, H, DH = 2048, 128, 4, 32
NCORES = 8
BS = B_FULL // NCORES          # 256
NTOK = 81
MAX_STEPS, THRESH = 16, 0.99
CH = 4                         # elements per chunk (324 tokens)
SG = 2                         # elements per scores/exp sub-group

_BUILD_CACHE = {}


# ----------------------------------------------------------------------------
# host-side helpers
# ----------------------------------------------------------------------------

def _f32(x):
    return np.ascontiguousarray(np.asarray(x), dtype=np.float32)


def _fold_weights(ins):
    """Fold conv/BN/LN affines into matmul weights. Returns device tensors."""
    W_row = _f32(ins["W_row"]).reshape(48, 90)   # (f, c*9+k)
    W_col = _f32(ins["W_col"]).reshape(48, 90)   # (f, c*9+row)
    W_box = _f32(ins["W_box"]).reshape(48, 90)   # (f, c*9+i*3+j)
    b_row, b_col, b_box = _f32(ins["b_row"]), _f32(ins["b_col"]), _f32(ins["b_box"])
    W_red, b_red = _f32(ins["W_red"]), _f32(ins["b_red"])
    bn_g, bn_b = _f32(ins["bn_g"]), _f32(ins["bn_b"])
    bn_m, bn_v = _f32(ins["bn_m"]), _f32(ins["bn_v"])

    Wr, Wc, Wb = W_red[:, :48], W_red[:, 48:96], W_red[:, 96:144]
    bs_ = bn_g / np.sqrt(bn_v + 1e-5)
    M_row = (bs_[:, None] * (Wr @ W_row))            # (128, 90)
    M_col = (bs_[:, None] * (Wc @ W_col))
    M_box = (bs_[:, None] * (Wb @ W_box))
    cb = Wr @ b_row + Wc @ b_col + Wb @ b_box + b_red
    bias0 = bs_ * cb + (bn_b - bn_m * bs_)           # (128,)

    W_qkv, b_qkv = _f32(ins["W_qkv"]), _f32(ins["b_qkv"])
    W_o, b_o = _f32(ins["W_o"]), _f32(ins["b_o"])
    ln1_g, ln1_b = _f32(ins["ln1_g"]), _f32(ins["ln1_b"])
    W_f1, b_f1 = _f32(ins["W_f1"]), _f32(ins["b_f1"])
    W_f2, b_f2 = _f32(ins["W_f2"]), _f32(ins["b_f2"])
    ln2_g, ln2_b = _f32(ins["ln2_g"]), _f32(ins["ln2_b"])
    W_h, b_h = _f32(ins["W_h"]), _f32(ins["b_h"])
    W_fc, b_fc = _f32(ins["W_fc"]), _f32(ins["b_fc"])

    sc = 1.0 / math.sqrt(DH)
    Wq, Wk, Wv = W_qkv[:D], W_qkv[D:2 * D], W_qkv[2 * D:]
    bq, bk, bv = b_qkv[:D], b_qkv[D:2 * D], b_qkv[2 * D:]

    out = {
        "mrowT": M_row.T.copy(), "mcolT": M_col.T.copy(), "mboxT": M_box.T.copy(),
        "bias0_row": bias0.reshape(1, 128).copy(),
    }
    # step 1 input is real s0; step 2 input is x-hat with affine (ln2_g, ln2_b)
    for s, (g_in, b_in) in (("1", (None, None)), ("2", (ln2_g, ln2_b))):
        def fold(W, b):
            if g_in is None:
                return W.copy(), b.copy()
            return W * g_in[None, :], b + W @ b_in
        Wq_s, bq_s = fold(Wq * sc, bq * sc)
        Wk_s, bk_s = fold(Wk, bk)
        Wv_s, bv_s = fold(Wv, bv)
        out[f"wqT_{s}"] = Wq_s.T.copy()
        out[f"wkT_{s}"] = Wk_s.T.copy()
        out[f"wvT_{s}"] = Wv_s.T.copy()
        out[f"bq_row_{s}"] = bq_s.reshape(1, 128).copy()
        out[f"bk_row_{s}"] = bk_s.reshape(1, 128).copy()
        out[f"bo_row_{s}"] = (b_o + W_o @ bv_s).reshape(1, 128).copy()
        out[f"diag_{s}"] = (np.eye(D, dtype=np.float32) if g_in is None
                            else np.diag(g_in).astype(np.float32))
    out["woT"] = W_o.T.copy()
    # ffn with ln1 fold (same both steps)
    W_f1p = W_f1 * ln1_g[None, :]
    b_f1p = b_f1 + W_f1 @ ln1_b
    out["wf1T"] = W_f1p.T.copy()                  # (128, 512)
    out["bf1_cols"] = b_f1p.reshape(4, 128).T.copy()  # (128, 4) col f = bias of tile f
    wf2t = W_f2.T.copy()                          # (512, 128)
    out["wf2T"] = np.concatenate([wf2t[k * 128:(k + 1) * 128, :] for k in range(4)],
                                 axis=1)          # (128, 4*128), block k = slice k
    out["diag_g1"] = np.diag(ln1_g).astype(np.float32)
    out["bw_row"] = (b_f2 + ln1_b).reshape(1, 128).copy()
    # halting path (ln2 fold, same both steps)
    out["wh_col"] = ((W_h[0] * ln2_g) / 81.0).reshape(128, 1).copy()
    out["neg_bh"] = np.array([[-(float(W_h[0] @ ln2_b) + float(b_h[0]))]],
                             dtype=np.float32)
    # epilogue
    Wfcp = W_fc * ln2_g[None, :]
    bfcp = b_fc + W_fc @ ln2_b
    out["wfcT"] = Wfcp.T.copy()                   # (128, 9)
    out["bfc_row"] = bfcp.reshape(1, 9).copy()
    # device constants (host-supplied so matmul operands have one DMA producer)
    out["ident"] = np.eye(128, dtype=np.float32)
    mh = np.zeros((81, 512), dtype=np.float32)
    for h in range(H):
        mh[:, h * 128 + 32 * h: h * 128 + 32 * h + 32] = 1.0
    out["maskh"] = mh
    return out


# ----------------------------------------------------------------------------
# bass program
# ----------------------------------------------------------------------------

def _build(bs):
    import concourse.bass as bass
    import concourse.bacc as bacc
    import concourse.tile as tile
    from concourse import mybir
    from contextlib import ExitStack

    FP32 = mybir.dt.float32
    FP32R = mybir.dt.float32r
    AF = mybir.ActivationFunctionType
    ALU = mybir.AluOpType

    T = bs * NTOK
    n_chunk = (bs + CH - 1) // CH

    def r_(ap):
        # fp32r is rejected by the walrus verifier on this toolchain;
        # keep the hook so a faster dtype can be swapped in later.
        return ap

    nc = bacc.Bacc()

    def param(name, shape):
        return nc.declare_dram_parameter(name, list(shape), FP32, isOutput=False)

    xrow2d = param("xrow2d", (bs, 810))
    xcol2d = param("xcol2d", (bs, 810))
    xbox2d = param("xbox2d", (bs, 810))
    names_128 = ["wqT_1", "wkT_1", "wvT_1", "wqT_2", "wkT_2", "wvT_2",
                 "woT", "diag_1", "diag_2", "diag_g1"]
    P = {n: param(n, (128, 128)) for n in names_128}
    P["wf1T"] = param("wf1T", (128, 512))
    P["wf2T"] = param("wf2T", (128, 512))
    P["mrowT"] = param("mrowT", (90, 128))
    P["mcolT"] = param("mcolT", (90, 128))
    P["mboxT"] = param("mboxT", (90, 128))
    P["bf1_cols"] = param("bf1_cols", (128, 4))
    P["wh_col"] = param("wh_col", (128, 1))
    P["wfcT"] = param("wfcT", (128, 9))
    rows = ["bias0_row", "bq_row_1", "bk_row_1", "bq_row_2", "bk_row_2",
            "bo_row_1", "bo_row_2", "bw_row"]
    for n in rows:
        P[n] = param(n, (1, 128))
    P["bfc_row"] = param("bfc_row", (1, 9))
    P["neg_bh"] = param("neg_bh", (1, 1))
    P["ident"] = param("ident", (128, 128))
    P["maskh"] = param("maskh", (81, 512))

    q_t = nc.declare_dram_parameter("q_t", [81, bs * 9], FP32, isOutput=True)
    aux = nc.declare_dram_parameter("aux", [1, bs + 8], FP32, isOutput=True)
    xh1_dram = nc.dram_tensor("xh1_spill", [128, T], FP32)

    def fv(t, dims, off=0):
        """free-dim view of an AP: keep partition dim, replace free dims."""
        return bass.AP(tensor=t.tensor, offset=t.offset + off,
                       ap=[t.ap[0]] + [list(d) for d in dims])

    with ExitStack() as ctx:
        tc = ctx.enter_context(tile.TileContext(nc))
        const = ctx.enter_context(tc.tile_pool(name="const", bufs=1))
        state = ctx.enter_context(tc.tile_pool(name="state", bufs=1))
        work = ctx.enter_context(tc.tile_pool(name="work", bufs=2))
        halt = ctx.enter_context(tc.tile_pool(name="halt", bufs=1))

        # ---- constants into SBUF ----
        W = {}
        for n, p in P.items():
            shp = list(p.shape)
            tl = const.tile(shp, FP32, tag=f"w_{n}")
            nc.sync.dma_start(out=tl[:, :], in_=p[:, :])
            W[n] = tl
        ident = W["ident"]
        maskh = W["maskh"]
        ones_row = const.tile([1, 512], FP32, tag="ones_row")
        nc.vector.memset(ones_row, 1.0)
        eps_col = const.tile([128, 1], FP32, tag="eps")
        nc.vector.memset(eps_col, 1e-5)
        # scratch psum for "touch" matmuls that absorb sync waits before
        # transposes (the fused transpose LDW struct has one wait slot).
        touch_ps = ctx.enter_context(
            tc.tile_pool(name="touch_ps", bufs=1, space="PSUM"))
        touch_out = touch_ps.tile([1, 512], FP32, tag="touch")

        def pe_touch(ap_2d):
            """Tiny matmul reading ap_2d so a following transpose needs no
            fresh wait for that operand's producer."""
            col = bass.AP(tensor=ap_2d.tensor, offset=ap_2d.offset,
                          ap=[ap_2d.ap[0], [1, 1]])
            nc.tensor.matmul(touch_out[:1, :1], col, col,
                             start=True, stop=True, skip_group_check=True)

        def transpose(out_ap, in_ap):
            pe_touch(in_ap)
            p = in_ap.partition_size()
            nc.tensor.transpose(out_ap, in_ap, ident[:p, :p])

        X = state.tile([128, T], FP32, tag="X")          # the state buffer
        q_sb = state.tile([81, bs * 9], FP32, tag="q_sb")
        aux_sb = halt.tile([1, bs + 8], FP32, tag="aux")
        nc.vector.memset(aux_sb, 0.0)

        # ================= prologue =================
        with tc.tile_pool(name="pro_ps", bufs=2, space="PSUM") as pro_ps, \
             tc.tile_pool(name="pro_sb", bufs=2) as pro_sb:
            # load the three host-rearranged x layouts; transposes then read
            # contiguous (hb, 90) slices (walrus: 1 free dim on PE operands).
            nhalf = (bs + 127) // 128
            Xrow = pro_sb.tile([90, bs * 9], FP32, tag="Xrow", bufs=1)
            Xcol = pro_sb.tile([90, bs * 9], FP32, tag="Xcol", bufs=1)
            Xbox = pro_sb.tile([90, bs * 9], FP32, tag="Xbox", bufs=1)
            for i in range(nhalf):
                hb = min(128, bs - i * 128)
                xr = pro_sb.tile([128, 810], FP32, tag="xr_sb")
                xc = pro_sb.tile([128, 810], FP32, tag="xc_sb")
                xb = pro_sb.tile([128, 810], FP32, tag="xb_sb")
                nc.sync.dma_start(out=xr[:hb, :], in_=xrow2d[i * 128:i * 128 + hb, :])
                nc.sync.dma_start(out=xc[:hb, :], in_=xcol2d[i * 128:i * 128 + hb, :])
                nc.sync.dma_start(out=xb[:hb, :], in_=xbox2d[i * 128:i * 128 + hb, :])
                for r in range(9):
                    pt = pro_ps.tile([90, 512], FP32, tag="tp")
                    transpose(pt[:, :hb], xr[:hb, r * 90:(r + 1) * 90])
                    nc.vector.tensor_copy(
                        fv(Xrow[:, :], [[9, hb]], off=i * 128 * 9 + r), pt[:, :hb])
                for w_ in range(9):
                    pt = pro_ps.tile([90, 512], FP32, tag="tp")
                    transpose(pt[:, :hb], xc[:hb, w_ * 90:(w_ + 1) * 90])
                    nc.vector.tensor_copy(
                        fv(Xcol[:, :], [[9, hb]], off=i * 128 * 9 + w_), pt[:, :hb])
                for rc in range(9):
                    pt = pro_ps.tile([90, 512], FP32, tag="tp")
                    transpose(pt[:, :hb], xb[:hb, rc * 90:(rc + 1) * 90])
                    nc.vector.tensor_copy(
                        fv(Xbox[:, :], [[9, hb]], off=i * 128 * 9 + rc), pt[:, :hb])
            # A-branch matmuls on the compact (b, pos) tensors, then the
            # 9x broadcasts happen as small elementwise adds per chunk.
            Arow = pro_sb.tile([128, bs * 9], FP32, tag="Arow", bufs=1)
            Acol = pro_sb.tile([128, bs * 9], FP32, tag="Acol", bufs=1)
            Abox = pro_sb.tile([128, bs * 9], FP32, tag="Abox", bufs=1)
            nbr = bs * 9
            for o0 in range(0, nbr, 512):
                on = min(512, nbr - o0)
                for nm, src_t, dst in (("mrowT", Xrow, Arow), ("mcolT", Xcol, Acol),
                                       ("mboxT", Xbox, Abox)):
                    ps_ = pro_ps.tile([128, 512], FP32, tag="aps", bufs=2)
                    if nm == "mrowT":
                        nc.tensor.matmul(r_(ps_[:, :on]), r_(W["bias0_row"][:, :]),
                                         r_(ones_row[:1, :on]),
                                         start=True, stop=False)
                    nc.tensor.matmul(r_(ps_[:, :on]), r_(W[nm][:, :]),
                                     src_t[:, o0:o0 + on],
                                     start=(nm != "mrowT"), stop=True)
                    nc.scalar.activation(dst[:, o0:o0 + on], ps_[:, :on], AF.Copy)
            # combine + relu per chunk
            for c in range(n_chunk):
                e0 = c * CH
                ce = min(CH, bs - e0)
                tw = ce * NTOK
                pre = pro_sb.tile([128, CH * NTOK], FP32, tag="pre", bufs=1)
                # pre[(e,r,c)] = Arow[(e,r)] + Acol[(e,c)]
                nc.vector.tensor_tensor(
                    pre[:, :tw],
                    fv(Arow[:, :], [[9, ce], [1, 9], [0, 9]], off=e0 * 9),
                    fv(Acol[:, :], [[9, ce], [0, 9], [1, 9]], off=e0 * 9),
                    ALU.add)
                pre2 = pro_sb.tile([128, CH * NTOK], FP32, tag="pre2", bufs=1)
                boxj = pro_sb.tile([128, CH * 27], FP32, tag="boxj", bufs=1)
                nc.gpsimd.tensor_copy(
                    boxj[:, :ce * 27],
                    fv(Abox[:, :], [[9, ce], [1, 9], [0, 3]], off=e0 * 9))
                for i in range(3):
                    nc.vector.tensor_tensor(
                        fv(pre2[:, :], [[81, ce], [27, 3], [1, 9]], off=9 * i),
                        fv(pre[:, :], [[81, ce], [27, 3], [1, 9]], off=9 * i),
                        fv(boxj[:, :], [[27, ce], [9, 3], [1, 9]]),
                        ALU.add)
                nc.scalar.activation(X[:, e0 * NTOK:e0 * NTOK + tw],
                                     pre2[:, :tw], AF.Relu)

        # ================= transformer steps =================
        sp1_row = halt.tile([1, bs], FP32, tag="sp1")
        rsp1_row = halt.tile([1, bs], FP32, tag="rsp1")

        def halting(step):
            """gf -> logit -> hp; update halting state. X holds x-hat."""
            gfs = halt.tile([128, bs], FP32, tag="gfs")
            nc.vector.tensor_reduce(gfs[:, :], fv(X[:, :], [[81, bs], [1, 81]]),
                                    axis=mybir.AxisListType.X, op=ALU.add)
            with tc.tile_pool(name=f"lg_ps{step}", bufs=1, space="PSUM") as lgp:
                lg = lgp.tile([1, 512 * ((bs + 511) // 512)], FP32, tag="lg")
                nc.tensor.matmul(lg[:1, :bs], W["wh_col"][:, :], gfs[:, :],
                                 start=True, stop=True)
                epx = halt.tile([1, bs], FP32, tag="epx")
                nc.scalar.activation(epx[:, :], lg[:1, :bs], AF.Exp,
                                     bias=W["neg_bh"][:1, :1], scale=-1.0)
            hp = halt.tile([1, bs], FP32, tag=f"hp{step}")
            nc.vector.tensor_scalar(hp[:, :], epx[:, :], 1.0, None, ALU.add)
            nc.vector.reciprocal(hp[:, :], hp[:, :])
            if step == 1:
                nc.vector.tensor_copy(sp1_row[:, :], hp[:, :])
                nc.vector.tensor_scalar(rsp1_row[:, :], hp[:, :], -1.0, 1.0,
                                        ALU.mult, ALU.add)
                nc.vector.tensor_reduce(aux_sb[:, bs:bs + 1], hp[:, :],
                                        axis=mybir.AxisListType.X, op=ALU.max)
            else:
                sp2 = halt.tile([1, bs], FP32, tag="sp2")
                nc.vector.tensor_tensor(sp2[:, :], hp[:, :], rsp1_row[:, :],
                                        ALU.min)
                nh2 = halt.tile([1, bs], FP32, tag="nh2")
                nc.vector.tensor_tensor(nh2[:, :], sp1_row[:, :], sp2[:, :],
                                        ALU.add)
                nc.vector.tensor_reduce(aux_sb[:, bs + 1:bs + 2], nh2[:, :],
                                        axis=mybir.AxisListType.X, op=ALU.max)
                rem = halt.tile([1, bs], FP32, tag="rem")
                nc.vector.tensor_scalar(rem[:, :], nh2[:, :], -1.0, 1.0,
                                        ALU.mult, ALU.add)
                ind = halt.tile([1, bs], FP32, tag="ind")
                nc.scalar.sign(ind[:, :], rem[:, :])
                nc.vector.tensor_scalar(ind[:, :], ind[:, :], 0.0, None, ALU.max)
                pond = halt.tile([1, bs], FP32, tag="pond")
                nc.vector.tensor_tensor(pond[:, :], ind[:, :], rem[:, :], ALU.add)
                nc.vector.tensor_scalar(aux_sb[:, 0:bs], pond[:, :], 1.0, None,
                                        ALU.add)

        def attention_block(step):
            s = str(step)
            with tc.tile_pool(name=f"a_ps{step}", bufs=1, space="PSUM") as ps1, \
                 tc.tile_pool(name=f"a_ps2{step}", bufs=2, space="PSUM") as ps2, \
                 tc.tile_pool(name=f"a_sb{step}", bufs=2) as asb:
                for c in range(n_chunk):
                    e0 = c * CH
                    ce = min(CH, bs - e0)
                    tw = ce * NTOK
                    t0 = e0 * NTOK
                    Xc = X[:, t0:t0 + tw]
                    # --- q, k ---
                    qps = ps2.tile([128, 512], FP32, tag="qk")
                    kps = ps2.tile([128, 512], FP32, tag="qk")
                    nc.tensor.matmul(r_(qps[:, :tw]), r_(W[f"bq_row_{s}"][:, :]),
                                     r_(ones_row[:1, :tw]), start=True, stop=False)
                    nc.tensor.matmul(r_(qps[:, :tw]), r_(W[f"wqT_{s}"][:, :]),
                                     r_(Xc), start=False, stop=True)
                    nc.tensor.matmul(r_(kps[:, :tw]), r_(W[f"bk_row_{s}"][:, :]),
                                     r_(ones_row[:1, :tw]), start=True, stop=False)
                    nc.tensor.matmul(r_(kps[:, :tw]), r_(W[f"wkT_{s}"][:, :]),
                                     r_(Xc), start=False, stop=True)
                    q_c = asb.tile([128, CH * NTOK], FP32, tag="q_c")
                    k_c = asb.tile([128, CH * NTOK], FP32, tag="k_c")
                    nc.vector.tensor_copy(q_c[:, :tw], qps[:, :tw])
                    nc.scalar.activation(k_c[:, :tw], kps[:, :tw], AF.Copy)
                    # --- v (token-partition layout) ---
                    vps = ps1.tile([81, 512], FP32, tag="acc", bufs=1)
                    for i in range(ce):
                        nc.tensor.matmul(
                            vps[:, i * 128:(i + 1) * 128],
                            X[:, t0 + i * NTOK: t0 + (i + 1) * NTOK],
                            W[f"wvT_{s}"][:, :], start=True, stop=True)
                    v_c = asb.tile([81, CH * 128], FP32, tag="v_c")
                    nc.scalar.activation(v_c[:, :ce * 128], vps[:, :ce * 128],
                                         AF.Copy)
                    # --- scores + exp; psum/e_c laid out (h, e, q) so the
                    # denb and attnv matmul rhs operands are 1-D slices ---
                    e_c = asb.tile([81, CH * 324], FP32, tag="e_c")
                    scp = ps1.tile([81, 2048], FP32, tag="scp", bufs=1)
                    for i in range(ce):
                        for h in range(H):
                            nc.tensor.matmul(
                                scp[:81, h * 512 + i * 81: h * 512 + i * 81 + 81],
                                k_c[32 * h:32 * h + 32, i * NTOK:(i + 1) * NTOK],
                                q_c[32 * h:32 * h + 32, i * NTOK:(i + 1) * NTOK],
                                start=True, stop=True,
                                tile_position=(32 * h, 0))
                    nc.scalar.activation(
                        fv(e_c[:, :], [[ce * 81, 4], [1, ce * 81]]),
                        fv(scp[:, :], [[512, 4], [1, ce * 81]]),
                        AF.Exp)
                    # --- denominators broadcast ---
                    dps = ps1.tile([128, 512], FP32, tag="acc", bufs=1)
                    for h in range(H):
                        nc.tensor.matmul(
                            r_(dps[:, :tw]), r_(maskh[:, h * 128:(h + 1) * 128]),
                            e_c[:81, h * ce * 81:(h + 1) * ce * 81],
                            start=(h == 0), stop=(h == H - 1))
                    rdb = asb.tile([128, CH * NTOK], FP32, tag="rdb")
                    nc.vector.reciprocal(rdb[:, :tw], dps[:, :tw])
                    # --- attn @ v (col-packed by head) ---
                    ops_ = ps1.tile([128, 512], FP32, tag="acc", bufs=1)
                    for i in range(ce):
                        for h in range(H):
                            nc.tensor.matmul(
                                ops_[32 * h:32 * h + 32, i * NTOK:(i + 1) * NTOK],
                                v_c[:81, i * 128 + 32 * h: i * 128 + 32 * h + 32],
                                e_c[:81, h * ce * 81 + i * 81: h * ce * 81 + (i + 1) * 81],
                                start=True, stop=True,
                                tile_position=(0, 32 * h))
                    o_c = asb.tile([128, CH * NTOK], FP32, tag="o_c")
                    nc.vector.tensor_tensor(o_c[:, :tw], ops_[:, :tw],
                                            rdb[:, :tw], ALU.mult)
                    # --- Wo + residual + bias ---
                    ups = ps1.tile([128, 512], FP32, tag="acc", bufs=1)
                    nc.tensor.matmul(r_(ups[:, :tw]), r_(W[f"bo_row_{s}"][:, :]),
                                     r_(ones_row[:1, :tw]), start=True, stop=False)
                    nc.tensor.matmul(r_(ups[:, :tw]), r_(W["woT"][:, :]),
                                     r_(o_c[:, :tw]), start=False, stop=False)
                    nc.tensor.matmul(r_(ups[:, :tw]), r_(W[f"diag_{s}"][:, :]),
                                     r_(Xc), start=False, stop=True)
                    nc.scalar.activation(Xc, ups[:, :tw], AF.Copy)

        def layernorm_block(step, tag):
            ntile = (T + 127) // 128
            SC = 4
            with tc.tile_pool(name=f"ln_ps_{tag}{step}", bufs=2, space="PSUM") as lps, \
                 tc.tile_pool(name=f"ln_sb_{tag}{step}", bufs=2) as lsb:
                for sc0 in range(0, ntile, SC):
                    scn = min(SC, ntile - sc0)
                    fwd = lps.tile([128, SC * 128], FP32, tag="fwd")
                    bwd = lps.tile([128, SC * 128], FP32, tag="bwd")
                    mv = lsb.tile([128, SC * 2], FP32, tag="mv")
                    rr = lsb.tile([128, SC], FP32, tag="rr")
                    nc.vector.memset(mv, 1.0)
                    sizes = []
                    for ti in range(scn):
                        t_ = sc0 + ti
                        w_ = min(128, T - t_ * 128)
                        sizes.append(w_)
                        transpose(fwd[:w_, ti * 128: ti * 128 + 128],
                                  X[:, t_ * 128: t_ * 128 + w_])
                        st6 = lsb.tile([128, 6], FP32, tag="st6")
                        nc.vector.bn_stats(st6[:w_, :],
                                           fwd[:w_, ti * 128: ti * 128 + 128])
                        nc.vector.bn_aggr(mv[:w_, ti * 2: ti * 2 + 2], st6[:w_, :])
                    wmax = max(sizes)
                    nc.scalar.activation(rr[:wmax, :scn],
                                         fv(mv[:wmax, :], [[2, scn]], off=1),
                                         AF.Ln, bias=eps_col[:wmax, :])
                    nc.scalar.activation(rr[:wmax, :scn], rr[:wmax, :scn],
                                         AF.Exp, scale=-0.5)
                    for ti in range(scn):
                        t_ = sc0 + ti
                        w_ = sizes[ti]
                        xh_t = lsb.tile([128, 128], FP32, tag="xh_t")
                        nc.vector.tensor_scalar(
                            xh_t[:w_, :], fwd[:w_, ti * 128: ti * 128 + 128],
                            mv[:w_, ti * 2: ti * 2 + 1],
                            rr[:w_, ti: ti + 1],
                            ALU.subtract, ALU.mult)
                        transpose(bwd[:, ti * 128: ti * 128 + w_],
                                  xh_t[:w_, :])
                    for ti in range(scn):
                        t_ = sc0 + ti
                        w_ = sizes[ti]
                        nc.scalar.activation(X[:, t_ * 128: t_ * 128 + w_],
                                             bwd[:, ti * 128: ti * 128 + w_],
                                             AF.Copy)

        def ffn_block(step):
            with tc.tile_pool(name=f"f_ps{step}", bufs=1, space="PSUM") as fps, \
                 tc.tile_pool(name=f"f_ps2{step}", bufs=2, space="PSUM") as fps2, \
                 tc.tile_pool(name=f"f_sb{step}", bufs=2) as fsb:
                for c in range(n_chunk):
                    e0 = c * CH
                    ce = min(CH, bs - e0)
                    tw = ce * NTOK
                    t0 = e0 * NTOK
                    Xc = X[:, t0:t0 + tw]
                    ff = fsb.tile([128, 4 * CH * NTOK], FP32, tag="ff")
                    for f in range(4):
                        fp = fps2.tile([128, 512], FP32, tag="fp")
                        nc.tensor.matmul(r_(fp[:, :tw]),
                                         r_(W["wf1T"][:, f * 128:(f + 1) * 128]),
                                         r_(Xc), start=True, stop=True)
                        nc.scalar.activation(
                            ff[:, f * tw: (f + 1) * tw], fp[:, :tw], AF.Relu,
                            bias=W["bf1_cols"][:, f:f + 1])
                    wp = fps.tile([128, 512], FP32, tag="wp")
                    nc.tensor.matmul(r_(wp[:, :tw]), r_(W["bw_row"][:, :]),
                                     r_(ones_row[:1, :tw]), start=True, stop=False)
                    for k in range(4):
                        nc.tensor.matmul(r_(wp[:, :tw]),
                                         r_(W["wf2T"][:, k * 128:(k + 1) * 128]),
                                         r_(ff[:, k * tw:(k + 1) * tw]),
                                         start=False, stop=False)
                    nc.tensor.matmul(r_(wp[:, :tw]), r_(W["diag_g1"][:, :]),
                                     r_(Xc), start=False, stop=True)
                    nc.scalar.activation(Xc, wp[:, :tw], AF.Copy)

        # ---- step 1 ----
        attention_block(1)
        layernorm_block(1, "a")
        ffn_block(1)
        layernorm_block(1, "b")
        halting(1)
        nc.sync.dma_start(out=xh1_dram[:, :], in_=X[:, :])

        # ---- step 2 ----
        attention_block(2)
        layernorm_block(2, "a")
        ffn_block(2)
        layernorm_block(2, "b")
        halting(2)

        # ================= epilogue =================
        with tc.tile_pool(name="e_ps", bufs=2, space="PSUM") as eps_ps, \
             tc.tile_pool(name="e_qp", bufs=1, space="PSUM") as eqp_ps, \
             tc.tile_pool(name="e_sb", bufs=2) as esb:
            wfc_sp = esb.tile([128, bs * 9], FP32, tag="wfc_sp", bufs=1)
            wfc_rem = esb.tile([128, bs * 9], FP32, tag="wfc_rem", bufs=1)
            sp1_9 = esb.tile([1, bs * 9], FP32, tag="sp1_9", bufs=1)
            nc.vector.tensor_copy(sp1_9[:1, :],
                                  fv(sp1_row[:1, :], [[1, bs], [0, 9]]))
            nw = bs * 9
            for o0 in range(0, nw, 504):
                on = min(504, nw - o0)
                e_lo = o0 // 9
                bc = eps_ps.tile([128, 512], FP32, tag="bc")
                nc.tensor.matmul(
                    r_(bc[:, :on]), r_(ones_row[:1, :128]),
                    sp1_9[:1, o0:o0 + on],
                    start=True, stop=True)
                nc.vector.tensor_tensor(
                    wfc_sp[:, o0:o0 + on],
                    fv(W["wfcT"][:, :], [[0, (on + 8) // 9], [1, 9]]),
                    bc[:, :on], ALU.mult)
            nc.gpsimd.tensor_tensor(
                wfc_rem[:, :], fv(W["wfcT"][:, :], [[0, bs], [1, 9]]),
                wfc_sp[:, :], ALU.subtract)
            # q-psums: 81 x 504 banks
            xh1_sb = None
            ngrp = (bs + 55) // 56
            qps_list = []
            bfc_rep = esb.tile([1, 504], FP32, tag="bfc_rep", bufs=1)
            nc.vector.tensor_copy(bfc_rep[:1, :],
                                  fv(W["bfc_row"][:1, :], [[0, 56], [1, 9]]))
            for g in range(ngrp):
                gqp = eqp_ps.tile([81, 512], FP32, tag=f"qp{g}")
                gn = min(56, bs - g * 56) * 9
                nc.tensor.matmul(
                    r_(gqp[:, :gn]), r_(ones_row[:1, :81]),
                    bfc_rep[:1, :gn],
                    start=True, stop=False)
                qps_list.append((gqp, gn))
            for c in range(n_chunk):
                e0 = c * CH
                ce = min(CH, bs - e0)
                tw = ce * NTOK
                xh1_sb = esb.tile([128, CH * NTOK], FP32, tag="xh1_sb")
                nc.sync.dma_start(out=xh1_sb[:, :tw],
                                  in_=xh1_dram[:, e0 * NTOK: e0 * NTOK + tw])
                for i in range(ce):
                    e = e0 + i
                    g, off = divmod(e, 56)
                    gqp, gn = qps_list[g]
                    last = e == bs - 1 or off == 55
                    nc.tensor.matmul(
                        gqp[:, off * 9: off * 9 + 9],
                        xh1_sb[:, i * NTOK:(i + 1) * NTOK],
                        wfc_sp[:, e * 9:(e + 1) * 9],
                        start=False, stop=False)
                    nc.tensor.matmul(
                        gqp[:, off * 9: off * 9 + 9],
                        X[:, e * NTOK:(e + 1) * NTOK],
                        wfc_rem[:, e * 9:(e + 1) * 9],
                        start=False, stop=last)
            for g in range(ngrp):
                gqp, gn = qps_list[g]
                nc.scalar.activation(q_sb[:, g * 504: g * 504 + gn],
                                     gqp[:, :gn], AF.Copy)
            nc.sync.dma_start(out=q_t[:, :], in_=q_sb[:, :])
            nc.sync.dma_start(out=aux[:, :], in_=aux_sb[:, :])

    nc.finalize()
    return nc


# ----------------------------------------------------------------------------
# host entry
# ----------------------------------------------------------------------------

def _numpy_forward(ins):
    """Reference-equivalent numpy fallback (never used for sane inputs)."""
    x = _f32(ins["x"])
    b_sz = x.shape[0]
    W_row = _f32(ins["W_row"]).reshape(48, 90)
    W_col = _f32(ins["W_col"]).reshape(48, 90)
    W_box = _f32(ins["W_box"]).reshape(48, 90)

    def _ln(t, g, b):
        m = t.mean(-1, keepdims=True)
        v = t.var(-1, keepdims=True)
        return (t - m) / np.sqrt(v + 1e-5) * g + b

    row = np.einsum("fk,bkr->bfr",
                    W_row, x.transpose(0, 2, 3, 1).reshape(b_sz, 81, 10)
                    .reshape(b_sz, 9, 9, 10).transpose(0, 3, 1, 2)
                    .reshape(b_sz, 10, 9, 9).reshape(b_sz, 90, 9)) \
        if False else None
    # straightforward implementation:
    xr = x.reshape(b_sz, 10, 9, 9)
    row = np.einsum("bcrk,fck->bfr", xr, W_row.reshape(48, 10, 9)) \
        + _f32(ins["b_row"])[None, :, None]
    col = np.einsum("bckw,fck->bfw", xr, W_col.reshape(48, 10, 9)) \
        + _f32(ins["b_col"])[None, :, None]
    xb = xr.reshape(b_sz, 10, 3, 3, 3, 3)
    box = np.einsum("bcRiCj,fcij->bfRC", xb,
                    W_box.reshape(48, 10, 3, 3)) \
        + _f32(ins["b_box"])[None, :, None, None]
    W_red, b_red = _f32(ins["W_red"]), _f32(ins["b_red"])
    Wr, Wc, Wb = W_red[:, :48], W_red[:, 48:96], W_red[:, 96:144]
    A = np.einsum("of,bfr->bor", Wr, row)
    Cc = np.einsum("of,bfw->bow", Wc, col)
    Xx = np.einsum("of,bfRC->boRC", Wb, box)
    h = (A[:, :, :, None] + Cc[:, :, None, :]
         + np.repeat(np.repeat(Xx, 3, 2), 3, 3) + b_red[None, :, None, None])
    bn_g, bn_b = _f32(ins["bn_g"]), _f32(ins["bn_b"])
    bn_m, bn_v = _f32(ins["bn_m"]), _f32(ins["bn_v"])
    sc_ = bn_g / np.sqrt(bn_v + 1e-5)
    h = h * sc_[None, :, None, None] + (bn_b - bn_m * sc_)[None, :, None, None]
    h = np.maximum(h, 0.0)
    W_qkv, b_qkv = _f32(ins["W_qkv"]), _f32(ins["b_qkv"])
    W_o, b_o = _f32(ins["W_o"]), _f32(ins["b_o"])
    W_f1, b_f1 = _f32(ins["W_f1"]), _f32(ins["b_f1"])
    W_f2, b_f2 = _f32(ins["W_f2"]), _f32(ins["b_f2"])
    W_h, b_h = _f32(ins["W_h"]), _f32(ins["b_h"])
    W_fc, b_fc = _f32(ins["W_fc"]), _f32(ins["b_fc"])
    ln1_g, ln1_b = _f32(ins["ln1_g"]), _f32(ins["ln1_b"])
    ln2_g, ln2_b = _f32(ins["ln2_g"]), _f32(ins["ln2_b"])

    def reasoning(s):
        t = s.reshape(b_sz, D, 81).transpose(0, 2, 1)
        qkv = t @ W_qkv.T + b_qkv
        q, k, v = qkv[..., :D], qkv[..., D:2 * D], qkv[..., 2 * D:]
        q = q.reshape(b_sz, 81, H, DH)
        k = k.reshape(b_sz, 81, H, DH)
        v = v.reshape(b_sz, 81, H, DH)
        s_ = np.einsum("bqhd,bkhd->bhqk", q, k) / np.sqrt(np.float32(DH))
        s_ = s_ - s_.max(-1, keepdims=True)
        e = np.exp(s_)
        attn = e / e.sum(-1, keepdims=True)
        o = np.einsum("bhqk,bkhd->bqhd", attn, v).reshape(b_sz, 81, D)
        t = _ln(t + (o @ W_o.T + b_o), ln1_g, ln1_b)
        ffv = np.maximum(t @ W_f1.T + b_f1, 0.0) @ W_f2.T + b_f2
        t = _ln(t + ffv, ln2_g, ln2_b)
        ns = t.transpose(0, 2, 1).reshape(b_sz, D, 9, 9)
        gf = ns.mean(axis=(2, 3))
        hp = 1.0 / (1.0 + np.exp(-(gf @ W_h.T + b_h)))
        return ns, hp

    s = h
    halt_v = np.zeros((b_sz, 1), np.float32)
    ponder = np.zeros((b_sz,), np.float32)
    ssum = np.zeros_like(h)
    for _ in range(MAX_STEPS):
        if float(halt_v.max()) >= THRESH:
            break
        ns, hp = reasoning(s)
        sp = np.minimum(hp, 1.0 - halt_v)
        nh = halt_v + sp
        ponder = ponder + (nh < 1.0).astype(np.float32).squeeze(-1)
        ssum = ssum + ns * sp[:, :, None, None]
        s, halt_v = ns, nh
    rem = 1.0 - halt_v
    ssum = ssum + s * rem[:, :, None, None]
    ponder = ponder + rem.squeeze(-1)
    final = ssum.transpose(0, 2, 3, 1).reshape(b_sz, 81, D)
    q_values = (final @ W_fc.T + b_fc).reshape(b_sz, -1)
    return q_values.astype(np.float32), ponder.astype(np.float32)


PROFILE = False
_LAST_EXEC_NS = None
_LAST_RESULTS = None


def kernel(**inputs):
    from concourse.bass_utils import run_bass_kernel_spmd

    x = _f32(inputs["x"])
    b_full = x.shape[0]
    if b_full % NCORES != 0 or x.shape[1:] != (10, 9, 9):
        return _numpy_forward(inputs)
    bs = b_full // NCORES
    folded = _fold_weights(inputs)

    key = bs
    if key not in _BUILD_CACHE:
        _BUILD_CACHE[key] = _build(bs)
    nc = _BUILD_CACHE[key]

    xrow2d = np.ascontiguousarray(x.transpose(0, 2, 1, 3).reshape(b_full, 810))
    xcol2d = np.ascontiguousarray(x.transpose(0, 3, 1, 2).reshape(b_full, 810))
    xbox2d = np.ascontiguousarray(
        x.reshape(b_full, 10, 3, 3, 3, 3).transpose(0, 2, 4, 1, 3, 5)
        .reshape(b_full, 810))
    in_maps = []
    for c in range(NCORES):
        m = dict(folded)
        sl = slice(c * bs, (c + 1) * bs)
        m["xrow2d"] = xrow2d[sl]
        m["xcol2d"] = xcol2d[sl]
        m["xbox2d"] = xbox2d[sl]
        in_maps.append(m)
    global _LAST_EXEC_NS, _LAST_RESULTS
    import time as _time
    _t0 = _time.monotonic()
    r = run_bass_kernel_spmd(nc, in_maps, list(range(NCORES)), trace=PROFILE)
    _wall_ns = int((_time.monotonic() - _t0) * 1e9)
    _LAST_EXEC_NS = r.exec_time_ns if r.exec_time_ns is not None else _wall_ns
    _LAST_RESULTS = r
    res = r.results

    q_parts, p_parts = [], []
    hm1 = -1e30
    hm2 = -1e30
    for c in range(NCORES):
        qt = res[c]["q_t"]                      # (81, bs*9)
        au = res[c]["aux"][0]                   # (bs+8,)
        q_parts.append(qt.reshape(81, bs, 9).transpose(1, 0, 2).reshape(bs, 729))
        p_parts.append(au[:bs])
        hm1 = max(hm1, float(au[bs]))
        hm2 = max(hm2, float(au[bs + 1]))
    if not (hm1 < THRESH <= hm2):
        # Halting pattern differs from the fused 2-step fast path: recompute
        # exactly on host (slow, should never trigger for this model family).
        return _numpy_forward(inputs)
    q_values = np.concatenate(q_parts, axis=0)
    ponder = np.concatenate(p_parts, axis=0)
    return q_values, ponder


if __name__ == "__main__":
    d = np.load("/root/problem/inputs_cache.npz")
    ins = {k: d[k] for k in d.files}
    q, p = kernel(**ins)
    print("q", q.shape, "ponder", p.shape)


# revision 36
# speedup vs baseline: 1.1247x; 1.1247x over previous
"""Trainium2 Bass kernel for nn_DQNSolverCNN6 (Sudoku DQN with ACT halting).

Self-contained: host-side weight folding + 8-core SPMD bass kernel.

Structure (per core, batch shard BS=256, tokens T=BS*81):
  prologue: conv branches folded into 3 (128x90) matmuls + BN + ReLU -> s0
  2 transformer steps (global halting freezes the net after step 2 for any
  realistic input; host validates via halt-max outputs and falls back to a
  numpy path if the input behaves differently)
  epilogue: q = Wfc @ (sp1*ns1 + (1-sp1)*ns2) + bias, ponder from hp1/hp2.

Layouts: activations (128 d-partitions, T tokens) fp32 in SBUF; attention
per-element with (j, q) score layout; LayerNorm via PE-transpose land.
"""
import math
import numpy as np

B_FULL, D, H, DH = 2048, 128, 4, 32
NCORES = 8
BS = B_FULL // NCORES          # 256
NTOK = 81
MAX_STEPS, THRESH = 16, 0.99
CH = 4                         # elements per chunk (324 tokens)
SG = 2                         # elements per scores/exp sub-group

_BUILD_CACHE = {}


# ----------------------------------------------------------------------------
# host-side helpers
# ----------------------------------------------------------------------------

def _f32(x):
    return np.ascontiguousarray(np.asarray(x), dtype=np.float32)


def _fold_weights(ins):
    """Fold conv/BN/LN affines into matmul weights. Returns device tensors."""
    W_row = _f32(ins["W_row"]).reshape(48, 90)   # (f, c*9+k)
    W_col = _f32(ins["W_col"]).reshape(48, 90)   # (f, c*9+row)
    W_box = _f32(ins["W_box"]).reshape(48, 90)   # (f, c*9+i*3+j)
    b_row, b_col, b_box = _f32(ins["b_row"]), _f32(ins["b_col"]), _f32(ins["b_box"])
    W_red, b_red = _f32(ins["W_red"]), _f32(ins["b_red"])
    bn_g, bn_b = _f32(ins["bn_g"]), _f32(ins["bn_b"])
    bn_m, bn_v = _f32(ins["bn_m"]), _f32(ins["bn_v"])

    Wr, Wc, Wb = W_red[:, :48], W_red[:, 48:96], W_red[:, 96:144]
    bs_ = bn_g / np.sqrt(bn_v + 1e-5)
    M_row = (bs_[:, None] * (Wr @ W_row))            # (128, 90)
    M_col = (bs_[:, None] * (Wc @ W_col))
    M_box = (bs_[:, None] * (Wb @ W_box))
    cb = Wr @ b_row + Wc @ b_col + Wb @ b_box + b_red
    bias0 = bs_ * cb + (bn_b - bn_m * bs_)           # (128,)

    W_qkv, b_qkv = _f32(ins["W_qkv"]), _f32(ins["b_qkv"])
    W_o, b_o = _f32(ins["W_o"]), _f32(ins["b_o"])
    ln1_g, ln1_b = _f32(ins["ln1_g"]), _f32(ins["ln1_b"])
    W_f1, b_f1 = _f32(ins["W_f1"]), _f32(ins["b_f1"])
    W_f2, b_f2 = _f32(ins["W_f2"]), _f32(ins["b_f2"])
    ln2_g, ln2_b = _f32(ins["ln2_g"]), _f32(ins["ln2_b"])
    W_h, b_h = _f32(ins["W_h"]), _f32(ins["b_h"])
    W_fc, b_fc = _f32(ins["W_fc"]), _f32(ins["b_fc"])

    sc = 1.0 / math.sqrt(DH)
    Wq, Wk, Wv = W_qkv[:D], W_qkv[D:2 * D], W_qkv[2 * D:]
    bq, bk, bv = b_qkv[:D], b_qkv[D:2 * D], b_qkv[2 * D:]

    out = {
        "mrowT": M_row.T.copy(), "mcolT": M_col.T.copy(), "mboxT": M_box.T.copy(),
        "bias0_row": bias0.reshape(1, 128).copy(),
    }
    # step 1 input is real s0; step 2 input is x-hat with affine (ln2_g, ln2_b)
    for s, (g_in, b_in) in (("1", (None, None)), ("2", (ln2_g, ln2_b))):
        def fold(W, b):
            if g_in is None:
                return W.copy(), b.copy()
            return W * g_in[None, :], b + W @ b_in
        Wq_s, bq_s = fold(Wq * sc, bq * sc)
        Wk_s, bk_s = fold(Wk, bk)
        Wv_s, bv_s = fold(Wv, bv)
        out[f"wqT_{s}"] = Wq_s.T.copy()
        out[f"wkT_{s}"] = Wk_s.T.copy()
        out[f"wvT_{s}"] = Wv_s.T.copy()
        out[f"bq_row_{s}"] = bq_s.reshape(1, 128).copy()
        out[f"bk_row_{s}"] = bk_s.reshape(1, 128).copy()
        out[f"bo_row_{s}"] = (b_o + W_o @ bv_s).reshape(1, 128).copy()
        out[f"diag_{s}"] = (np.eye(D, dtype=np.float32) if g_in is None
                            else np.diag(g_in).astype(np.float32))
    out["woT"] = W_o.T.copy()
    # ffn with ln1 fold (same both steps)
    W_f1p = W_f1 * ln1_g[None, :]
    b_f1p = b_f1 + W_f1 @ ln1_b
    out["wf1T"] = W_f1p.T.copy()                  # (128, 512)
    out["bf1_cols"] = b_f1p.reshape(4, 128).T.copy()  # (128, 4) col f = bias of tile f
    wf2t = W_f2.T.copy()                          # (512, 128)
    out["wf2T"] = np.concatenate([wf2t[k * 128:(k + 1) * 128, :] for k in range(4)],
                                 axis=1)          # (128, 4*128), block k = slice k
    out["diag_g1"] = np.diag(ln1_g).astype(np.float32)
    out["bw_row"] = (b_f2 + ln1_b).reshape(1, 128).copy()
    # halting path (ln2 fold, same both steps)
    out["wh_col"] = ((W_h[0] * ln2_g) / 81.0).reshape(128, 1).copy()
    out["neg_bh"] = np.array([[-(float(W_h[0] @ ln2_b) + float(b_h[0]))]],
                             dtype=np.float32)
    # epilogue
    Wfcp = W_fc * ln2_g[None, :]
    bfcp = b_fc + W_fc @ ln2_b
    out["wfcT"] = Wfcp.T.copy()                   # (128, 9)
    out["bfc_row"] = bfcp.reshape(1, 9).copy()
    # device constants (host-supplied so matmul operands have one DMA producer)
    out["ident"] = np.eye(128, dtype=np.float32)
    mh = np.zeros((81, 512), dtype=np.float32)
    for h in range(H):
        mh[:, h * 128 + 32 * h: h * 128 + 32 * h + 32] = 1.0
    out["maskh"] = mh
    return out


# ----------------------------------------------------------------------------
# bass program
# ----------------------------------------------------------------------------

def _build(bs):
    import concourse.bass as bass
    import concourse.bacc as bacc
    import concourse.tile as tile
    from concourse import mybir
    from contextlib import ExitStack

    FP32 = mybir.dt.float32
    FP32R = mybir.dt.float32r
    AF = mybir.ActivationFunctionType
    ALU = mybir.AluOpType

    T = bs * NTOK
    n_chunk = (bs + CH - 1) // CH

    def r_(ap):
        # fp32r is rejected by the walrus verifier on this toolchain;
        # keep the hook so a faster dtype can be swapped in later.
        return ap

    nc = bacc.Bacc()

    def param(name, shape):
        return nc.declare_dram_parameter(name, list(shape), FP32, isOutput=False)

    xrow2d = param("xrow2d", (bs, 810))
    xcol2d = param("xcol2d", (bs, 810))
    xbox2d = param("xbox2d", (bs, 810))
    names_128 = ["wqT_1", "wkT_1", "wvT_1", "wqT_2", "wkT_2", "wvT_2",
                 "woT", "diag_1", "diag_2", "diag_g1"]
    P = {n: param(n, (128, 128)) for n in names_128}
    P["wf1T"] = param("wf1T", (128, 512))
    P["wf2T"] = param("wf2T", (128, 512))
    P["mrowT"] = param("mrowT", (90, 128))
    P["mcolT"] = param("mcolT", (90, 128))
    P["mboxT"] = param("mboxT", (90, 128))
    P["bf1_cols"] = param("bf1_cols", (128, 4))
    P["wh_col"] = param("wh_col", (128, 1))
    P["wfcT"] = param("wfcT", (128, 9))
    rows = ["bias0_row", "bq_row_1", "bk_row_1", "bq_row_2", "bk_row_2",
            "bo_row_1", "bo_row_2", "bw_row"]
    for n in rows:
        P[n] = param(n, (1, 128))
    P["bfc_row"] = param("bfc_row", (1, 9))
    P["neg_bh"] = param("neg_bh", (1, 1))
    P["ident"] = param("ident", (128, 128))
    P["maskh"] = param("maskh", (81, 512))

    q_t = nc.declare_dram_parameter("q_t", [81, bs * 9], FP32, isOutput=True)
    aux = nc.declare_dram_parameter("aux", [1, bs + 8], FP32, isOutput=True)
    xh1_dram = nc.dram_tensor("xh1_spill", [128, T], FP32)

    def fv(t, dims, off=0):
        """free-dim view of an AP: keep partition dim, replace free dims."""
        return bass.AP(tensor=t.tensor, offset=t.offset + off,
                       ap=[t.ap[0]] + [list(d) for d in dims])

    with ExitStack() as ctx:
        tc = ctx.enter_context(tile.TileContext(nc))
        const = ctx.enter_context(tc.tile_pool(name="const", bufs=1))
        state = ctx.enter_context(tc.tile_pool(name="state", bufs=1))
        work = ctx.enter_context(tc.tile_pool(name="work", bufs=2))
        halt = ctx.enter_context(tc.tile_pool(name="halt", bufs=1))

        # ---- constants into SBUF ----
        W = {}
        for n, p in P.items():
            shp = list(p.shape)
            tl = const.tile(shp, FP32, tag=f"w_{n}")
            nc.sync.dma_start(out=tl[:, :], in_=p[:, :])
            W[n] = tl
        ident = W["ident"]
        maskh = W["maskh"]
        ones_row = const.tile([1, 512], FP32, tag="ones_row")
        nc.vector.memset(ones_row, 1.0)
        eps_col = const.tile([128, 1], FP32, tag="eps")
        nc.vector.memset(eps_col, 1e-5)
        # scratch psum for "touch" matmuls that absorb sync waits before
        # transposes (the fused transpose LDW struct has one wait slot).
        touch_ps = ctx.enter_context(
            tc.tile_pool(name="touch_ps", bufs=1, space="PSUM"))
        touch_out = touch_ps.tile([1, 512], FP32, tag="touch")

        def pe_touch(ap_2d):
            """Tiny matmul reading ap_2d so a following transpose needs no
            fresh wait for that operand's producer."""
            col = bass.AP(tensor=ap_2d.tensor, offset=ap_2d.offset,
                          ap=[ap_2d.ap[0], [1, 1]])
            nc.tensor.matmul(touch_out[:1, :1], col, col,
                             start=True, stop=True, skip_group_check=True)

        def transpose(out_ap, in_ap):
            # Bacc.generate_event_semaphores splits multi-waits, so the
            # pe_touch wait-absorber is no longer needed per transpose.
            p = in_ap.partition_size()
            nc.tensor.transpose(out_ap, in_ap, ident[:p, :p])

        X = state.tile([128, T], FP32, tag="X")          # the state buffer
        q_sb = state.tile([81, bs * 9], FP32, tag="q_sb")
        aux_sb = halt.tile([1, bs + 8], FP32, tag="aux")
        nc.vector.memset(aux_sb, 0.0)

        # ================= prologue =================
        with tc.tile_pool(name="pro_ps", bufs=2, space="PSUM") as pro_ps, \
             tc.tile_pool(name="pro_sb", bufs=2) as pro_sb:
            # load the three host-rearranged x layouts; transposes then read
            # contiguous (hb, 90) slices (walrus: 1 free dim on PE operands).
            nhalf = (bs + 127) // 128
            Xrow = pro_sb.tile([90, bs * 9], FP32, tag="Xrow", bufs=1)
            Xcol = pro_sb.tile([90, bs * 9], FP32, tag="Xcol", bufs=1)
            Xbox = pro_sb.tile([90, bs * 9], FP32, tag="Xbox", bufs=1)
            for i in range(nhalf):
                hb = min(128, bs - i * 128)
                xr = pro_sb.tile([128, 810], FP32, tag="xr_sb")
                xc = pro_sb.tile([128, 810], FP32, tag="xc_sb")
                xb = pro_sb.tile([128, 810], FP32, tag="xb_sb")
                nc.sync.dma_start(out=xr[:hb, :], in_=xrow2d[i * 128:i * 128 + hb, :])
                nc.sync.dma_start(out=xc[:hb, :], in_=xcol2d[i * 128:i * 128 + hb, :])
                nc.sync.dma_start(out=xb[:hb, :], in_=xbox2d[i * 128:i * 128 + hb, :])
                for r in range(9):
                    pt = pro_ps.tile([90, 512], FP32, tag="tp")
                    transpose(pt[:, :hb], xr[:hb, r * 90:(r + 1) * 90])
                    nc.vector.tensor_copy(
                        fv(Xrow[:, :], [[9, hb]], off=i * 128 * 9 + r), pt[:, :hb])
                for w_ in range(9):
                    pt = pro_ps.tile([90, 512], FP32, tag="tp")
                    transpose(pt[:, :hb], xc[:hb, w_ * 90:(w_ + 1) * 90])
                    nc.vector.tensor_copy(
                        fv(Xcol[:, :], [[9, hb]], off=i * 128 * 9 + w_), pt[:, :hb])
                for rc in range(9):
                    pt = pro_ps.tile([90, 512], FP32, tag="tp")
                    transpose(pt[:, :hb], xb[:hb, rc * 90:(rc + 1) * 90])
                    nc.vector.tensor_copy(
                        fv(Xbox[:, :], [[9, hb]], off=i * 128 * 9 + rc), pt[:, :hb])
            # A-branch matmuls on the compact (b, pos) tensors, then the
            # 9x broadcasts happen as small elementwise adds per chunk.
            Arow = pro_sb.tile([128, bs * 9], FP32, tag="Arow", bufs=1)
            Acol = pro_sb.tile([128, bs * 9], FP32, tag="Acol", bufs=1)
            Abox = pro_sb.tile([128, bs * 9], FP32, tag="Abox", bufs=1)
            nbr = bs * 9
            for o0 in range(0, nbr, 512):
                on = min(512, nbr - o0)
                for nm, src_t, dst in (("mrowT", Xrow, Arow), ("mcolT", Xcol, Acol),
                                       ("mboxT", Xbox, Abox)):
                    ps_ = pro_ps.tile([128, 512], FP32, tag="aps", bufs=2)
                    if nm == "mrowT":
                        nc.tensor.matmul(r_(ps_[:, :on]), r_(W["bias0_row"][:, :]),
                                         r_(ones_row[:1, :on]),
                                         start=True, stop=False)
                    nc.tensor.matmul(r_(ps_[:, :on]), r_(W[nm][:, :]),
                                     src_t[:, o0:o0 + on],
                                     start=(nm != "mrowT"), stop=True)
                    nc.scalar.activation(dst[:, o0:o0 + on], ps_[:, :on], AF.Copy)
            # combine + relu per chunk
            for c in range(n_chunk):
                e0 = c * CH
                ce = min(CH, bs - e0)
                tw = ce * NTOK
                pre = pro_sb.tile([128, CH * NTOK], FP32, tag="pre", bufs=1)
                # pre[(e,r,c)] = Arow[(e,r)] + Acol[(e,c)]
                nc.vector.tensor_tensor(
                    pre[:, :tw],
                    fv(Arow[:, :], [[9, ce], [1, 9], [0, 9]], off=e0 * 9),
                    fv(Acol[:, :], [[9, ce], [0, 9], [1, 9]], off=e0 * 9),
                    ALU.add)
                pre2 = pro_sb.tile([128, CH * NTOK], FP32, tag="pre2", bufs=1)
                boxj = pro_sb.tile([128, CH * 27], FP32, tag="boxj", bufs=1)
                nc.gpsimd.tensor_copy(
                    boxj[:, :ce * 27],
                    fv(Abox[:, :], [[9, ce], [1, 9], [0, 3]], off=e0 * 9))
                for i in range(3):
                    nc.vector.tensor_tensor(
                        fv(pre2[:, :], [[81, ce], [27, 3], [1, 9]], off=9 * i),
                        fv(pre[:, :], [[81, ce], [27, 3], [1, 9]], off=9 * i),
                        fv(boxj[:, :], [[27, ce], [9, 3], [1, 9]]),
                        ALU.add)
                nc.scalar.activation(X[:, e0 * NTOK:e0 * NTOK + tw],
                                     pre2[:, :tw], AF.Relu)

        # ================= transformer steps =================
        sp1_row = halt.tile([1, bs], FP32, tag="sp1")
        rsp1_row = halt.tile([1, bs], FP32, tag="rsp1")

        def halting(step):
            """gf -> logit -> hp; update halting state. X holds x-hat."""
            gfs = halt.tile([128, bs], FP32, tag="gfs")
            nc.vector.tensor_reduce(gfs[:, :], fv(X[:, :], [[81, bs], [1, 81]]),
                                    axis=mybir.AxisListType.X, op=ALU.add)
            with tc.tile_pool(name=f"lg_ps{step}", bufs=1, space="PSUM") as lgp:
                lg = lgp.tile([1, 512 * ((bs + 511) // 512)], FP32, tag="lg")
                nc.tensor.matmul(lg[:1, :bs], W["wh_col"][:, :], gfs[:, :],
                                 start=True, stop=True)
                epx = halt.tile([1, bs], FP32, tag="epx")
                nc.scalar.activation(epx[:, :], lg[:1, :bs], AF.Exp,
                                     bias=W["neg_bh"][:1, :1], scale=-1.0)
            hp = halt.tile([1, bs], FP32, tag=f"hp{step}")
            nc.vector.tensor_scalar(hp[:, :], epx[:, :], 1.0, None, ALU.add)
            nc.vector.reciprocal(hp[:, :], hp[:, :])
            if step == 1:
                nc.vector.tensor_copy(sp1_row[:, :], hp[:, :])
                nc.vector.tensor_scalar(rsp1_row[:, :], hp[:, :], -1.0, 1.0,
                                        ALU.mult, ALU.add)
                nc.vector.tensor_reduce(aux_sb[:, bs:bs + 1], hp[:, :],
                                        axis=mybir.AxisListType.X, op=ALU.max)
            else:
                sp2 = halt.tile([1, bs], FP32, tag="sp2")
                nc.vector.tensor_tensor(sp2[:, :], hp[:, :], rsp1_row[:, :],
                                        ALU.min)
                nh2 = halt.tile([1, bs], FP32, tag="nh2")
                nc.vector.tensor_tensor(nh2[:, :], sp1_row[:, :], sp2[:, :],
                                        ALU.add)
                nc.vector.tensor_reduce(aux_sb[:, bs + 1:bs + 2], nh2[:, :],
                                        axis=mybir.AxisListType.X, op=ALU.max)
                rem = halt.tile([1, bs], FP32, tag="rem")
                nc.vector.tensor_scalar(rem[:, :], nh2[:, :], -1.0, 1.0,
                                        ALU.mult, ALU.add)
                ind = halt.tile([1, bs], FP32, tag="ind")
                nc.scalar.sign(ind[:, :], rem[:, :])
                nc.vector.tensor_scalar(ind[:, :], ind[:, :], 0.0, None, ALU.max)
                pond = halt.tile([1, bs], FP32, tag="pond")
                nc.vector.tensor_tensor(pond[:, :], ind[:, :], rem[:, :], ALU.add)
                nc.vector.tensor_scalar(aux_sb[:, 0:bs], pond[:, :], 1.0, None,
                                        ALU.add)

        def attention_block(step):
            s = str(step)
            with tc.tile_pool(name=f"a_ps{step}", bufs=1, space="PSUM") as ps1, \
                 tc.tile_pool(name=f"a_ps2{step}", bufs=2, space="PSUM") as ps2, \
                 tc.tile_pool(name=f"a_sb{step}", bufs=2) as asb:
                for c in range(n_chunk):
                    e0 = c * CH
                    ce = min(CH, bs - e0)
                    tw = ce * NTOK
                    t0 = e0 * NTOK
                    Xc = X[:, t0:t0 + tw]
                    # --- q, k ---
                    qps = ps2.tile([128, 512], FP32, tag="qk")
                    kps = ps2.tile([128, 512], FP32, tag="qk")
                    nc.tensor.matmul(r_(qps[:, :tw]), r_(W[f"bq_row_{s}"][:, :]),
                                     r_(ones_row[:1, :tw]), start=True, stop=False)
                    nc.tensor.matmul(r_(qps[:, :tw]), r_(W[f"wqT_{s}"][:, :]),
                                     r_(Xc), start=False, stop=True)
                    nc.tensor.matmul(r_(kps[:, :tw]), r_(W[f"bk_row_{s}"][:, :]),
                                     r_(ones_row[:1, :tw]), start=True, stop=False)
                    nc.tensor.matmul(r_(kps[:, :tw]), r_(W[f"wkT_{s}"][:, :]),
                                     r_(Xc), start=False, stop=True)
                    q_c = asb.tile([128, CH * NTOK], FP32, tag="q_c")
                    k_c = asb.tile([128, CH * NTOK], FP32, tag="k_c")
                    nc.vector.tensor_copy(q_c[:, :tw], qps[:, :tw])
                    nc.scalar.activation(k_c[:, :tw], kps[:, :tw], AF.Copy)
                    # --- v (token-partition layout) ---
                    vps = ps1.tile([81, 512], FP32, tag="acc", bufs=1)
                    for i in range(ce):
                        nc.tensor.matmul(
                            vps[:, i * 128:(i + 1) * 128],
                            X[:, t0 + i * NTOK: t0 + (i + 1) * NTOK],
                            W[f"wvT_{s}"][:, :], start=True, stop=True)
                    v_c = asb.tile([81, CH * 128], FP32, tag="v_c")
                    nc.scalar.activation(v_c[:, :ce * 128], vps[:, :ce * 128],
                                         AF.Copy)
                    # --- scores + exp; psum/e_c laid out (h, e, q) so the
                    # denb and attnv matmul rhs operands are 1-D slices ---
                    e_c = asb.tile([81, CH * 324], FP32, tag="e_c")
                    scp = ps1.tile([81, 2048], FP32, tag="scp", bufs=1)
                    for i in range(ce):
                        for h in range(H):
                            nc.tensor.matmul(
                                scp[:81, h * 512 + i * 81: h * 512 + i * 81 + 81],
                                k_c[32 * h:32 * h + 32, i * NTOK:(i + 1) * NTOK],
                                q_c[32 * h:32 * h + 32, i * NTOK:(i + 1) * NTOK],
                                start=True, stop=True,
                                tile_position=(32 * h, 0))
                    nc.scalar.activation(
                        fv(e_c[:, :], [[ce * 81, 4], [1, ce * 81]]),
                        fv(scp[:, :], [[512, 4], [1, ce * 81]]),
                        AF.Exp)
                    # --- denominators broadcast ---
                    dps = ps1.tile([128, 512], FP32, tag="acc", bufs=1)
                    for h in range(H):
                        nc.tensor.matmul(
                            r_(dps[:, :tw]), r_(maskh[:, h * 128:(h + 1) * 128]),
                            e_c[:81, h * ce * 81:(h + 1) * ce * 81],
                            start=(h == 0), stop=(h == H - 1))
                    rdb = asb.tile([128, CH * NTOK], FP32, tag="rdb")
                    nc.vector.reciprocal(rdb[:, :tw], dps[:, :tw])
                    # --- attn @ v (col-packed by head) ---
                    ops_ = ps1.tile([128, 512], FP32, tag="acc", bufs=1)
                    for i in range(ce):
                        for h in range(H):
                            nc.tensor.matmul(
                                ops_[32 * h:32 * h + 32, i * NTOK:(i + 1) * NTOK],
                                v_c[:81, i * 128 + 32 * h: i * 128 + 32 * h + 32],
                                e_c[:81, h * ce * 81 + i * 81: h * ce * 81 + (i + 1) * 81],
                                start=True, stop=True,
                                tile_position=(0, 32 * h))
                    o_c = asb.tile([128, CH * NTOK], FP32, tag="o_c")
                    nc.vector.tensor_tensor(o_c[:, :tw], ops_[:, :tw],
                                            rdb[:, :tw], ALU.mult)
                    # --- Wo + residual + bias ---
                    ups = ps1.tile([128, 512], FP32, tag="acc", bufs=1)
                    nc.tensor.matmul(r_(ups[:, :tw]), r_(W[f"bo_row_{s}"][:, :]),
                                     r_(ones_row[:1, :tw]), start=True, stop=False)
                    nc.tensor.matmul(r_(ups[:, :tw]), r_(W["woT"][:, :]),
                                     r_(o_c[:, :tw]), start=False, stop=False)
                    nc.tensor.matmul(r_(ups[:, :tw]), r_(W[f"diag_{s}"][:, :]),
                                     r_(Xc), start=False, stop=True)
                    nc.scalar.activation(Xc, ups[:, :tw], AF.Copy)

        def layernorm_block(step, tag):
            ntile = (T + 127) // 128
            SC = 4
            with tc.tile_pool(name=f"ln_ps_{tag}{step}", bufs=2, space="PSUM") as lps, \
                 tc.tile_pool(name=f"ln_sb_{tag}{step}", bufs=2) as lsb:
                for sc0 in range(0, ntile, SC):
                    scn = min(SC, ntile - sc0)
                    fwd = lps.tile([128, SC * 128], FP32, tag="fwd")
                    bwd = lps.tile([128, SC * 128], FP32, tag="bwd")
                    mv = lsb.tile([128, SC * 2], FP32, tag="mv")
                    rr = lsb.tile([128, SC], FP32, tag="rr")
                    nc.vector.memset(mv, 1.0)
                    sizes = []
                    for ti in range(scn):
                        t_ = sc0 + ti
                        w_ = min(128, T - t_ * 128)
                        sizes.append(w_)
                        transpose(fwd[:w_, ti * 128: ti * 128 + 128],
                                  X[:, t_ * 128: t_ * 128 + w_])
                        st6 = lsb.tile([128, 6], FP32, tag="st6")
                        nc.vector.bn_stats(st6[:w_, :],
                                           fwd[:w_, ti * 128: ti * 128 + 128])
                        nc.vector.bn_aggr(mv[:w_, ti * 2: ti * 2 + 2], st6[:w_, :])
                    wmax = max(sizes)
                    nc.scalar.activation(rr[:wmax, :scn],
                                         fv(mv[:wmax, :], [[2, scn]], off=1),
                                         AF.Ln, bias=eps_col[:wmax, :])
                    nc.scalar.activation(rr[:wmax, :scn], rr[:wmax, :scn],
                                         AF.Exp, scale=-0.5)
                    for ti in range(scn):
                        t_ = sc0 + ti
                        w_ = sizes[ti]
                        xh_t = lsb.tile([128, 128], FP32, tag="xh_t")
                        nc.vector.tensor_scalar(
                            xh_t[:w_, :], fwd[:w_, ti * 128: ti * 128 + 128],
                            mv[:w_, ti * 2: ti * 2 + 1],
                            rr[:w_, ti: ti + 1],
                            ALU.subtract, ALU.mult)
                        transpose(bwd[:, ti * 128: ti * 128 + w_],
                                  xh_t[:w_, :])
                    for ti in range(scn):
                        t_ = sc0 + ti
                        w_ = sizes[ti]
                        nc.scalar.activation(X[:, t_ * 128: t_ * 128 + w_],
                                             bwd[:, ti * 128: ti * 128 + w_],
                                             AF.Copy)

        def ffn_block(step):
            with tc.tile_pool(name=f"f_ps{step}", bufs=1, space="PSUM") as fps, \
                 tc.tile_pool(name=f"f_ps2{step}", bufs=2, space="PSUM") as fps2, \
                 tc.tile_pool(name=f"f_sb{step}", bufs=2) as fsb:
                for c in range(n_chunk):
                    e0 = c * CH
                    ce = min(CH, bs - e0)
                    tw = ce * NTOK
                    t0 = e0 * NTOK
                    Xc = X[:, t0:t0 + tw]
                    ff = fsb.tile([128, 4 * CH * NTOK], FP32, tag="ff")
                    for f in range(4):
                        fp = fps2.tile([128, 512], FP32, tag="fp")
                        nc.tensor.matmul(r_(fp[:, :tw]),
                                         r_(W["wf1T"][:, f * 128:(f + 1) * 128]),
                                         r_(Xc), start=True, stop=True)
                        nc.scalar.activation(
                            ff[:, f * tw: (f + 1) * tw], fp[:, :tw], AF.Relu,
                            bias=W["bf1_cols"][:, f:f + 1])
                    wp = fps.tile([128, 512], FP32, tag="wp")
                    nc.tensor.matmul(r_(wp[:, :tw]), r_(W["bw_row"][:, :]),
                                     r_(ones_row[:1, :tw]), start=True, stop=False)
                    for k in range(4):
                        nc.tensor.matmul(r_(wp[:, :tw]),
                                         r_(W["wf2T"][:, k * 128:(k + 1) * 128]),
                                         r_(ff[:, k * tw:(k + 1) * tw]),
                                         start=False, stop=False)
                    nc.tensor.matmul(r_(wp[:, :tw]), r_(W["diag_g1"][:, :]),
                                     r_(Xc), start=False, stop=True)
                    nc.scalar.activation(Xc, wp[:, :tw], AF.Copy)

        # ---- step 1 ----
        attention_block(1)
        layernorm_block(1, "a")
        ffn_block(1)
        layernorm_block(1, "b")
        halting(1)
        nc.sync.dma_start(out=xh1_dram[:, :], in_=X[:, :])

        # ---- step 2 ----
        attention_block(2)
        layernorm_block(2, "a")
        ffn_block(2)
        layernorm_block(2, "b")
        halting(2)

        # ================= epilogue =================
        with tc.tile_pool(name="e_ps", bufs=2, space="PSUM") as eps_ps, \
             tc.tile_pool(name="e_qp", bufs=1, space="PSUM") as eqp_ps, \
             tc.tile_pool(name="e_sb", bufs=2) as esb:
            wfc_sp = esb.tile([128, bs * 9], FP32, tag="wfc_sp", bufs=1)
            wfc_rem = esb.tile([128, bs * 9], FP32, tag="wfc_rem", bufs=1)
            sp1_9 = esb.tile([1, bs * 9], FP32, tag="sp1_9", bufs=1)
            nc.vector.tensor_copy(sp1_9[:1, :],
                                  fv(sp1_row[:1, :], [[1, bs], [0, 9]]))
            nw = bs * 9
            for o0 in range(0, nw, 504):
                on = min(504, nw - o0)
                e_lo = o0 // 9
                bc = eps_ps.tile([128, 512], FP32, tag="bc")
                nc.tensor.matmul(
                    r_(bc[:, :on]), r_(ones_row[:1, :128]),
                    sp1_9[:1, o0:o0 + on],
                    start=True, stop=True)
                nc.vector.tensor_tensor(
                    wfc_sp[:, o0:o0 + on],
                    fv(W["wfcT"][:, :], [[0, (on + 8) // 9], [1, 9]]),
                    bc[:, :on], ALU.mult)
            nc.gpsimd.tensor_tensor(
                wfc_rem[:, :], fv(W["wfcT"][:, :], [[0, bs], [1, 9]]),
                wfc_sp[:, :], ALU.subtract)
            # q-psums: 81 x 504 banks
            xh1_sb = None
            ngrp = (bs + 55) // 56
            qps_list = []
            bfc_rep = esb.tile([1, 504], FP32, tag="bfc_rep", bufs=1)
            nc.vector.tensor_copy(bfc_rep[:1, :],
                                  fv(W["bfc_row"][:1, :], [[0, 56], [1, 9]]))
            for g in range(ngrp):
                gqp = eqp_ps.tile([81, 512], FP32, tag=f"qp{g}")
                gn = min(56, bs - g * 56) * 9
                nc.tensor.matmul(
                    r_(gqp[:, :gn]), r_(ones_row[:1, :81]),
                    bfc_rep[:1, :gn],
                    start=True, stop=False)
                qps_list.append((gqp, gn))
            for c in range(n_chunk):
                e0 = c * CH
                ce = min(CH, bs - e0)
                tw = ce * NTOK
                xh1_sb = esb.tile([128, CH * NTOK], FP32, tag="xh1_sb")
                nc.sync.dma_start(out=xh1_sb[:, :tw],
                                  in_=xh1_dram[:, e0 * NTOK: e0 * NTOK + tw])
                for i in range(ce):
                    e = e0 + i
                    g, off = divmod(e, 56)
                    gqp, gn = qps_list[g]
                    last = e == bs - 1 or off == 55
                    nc.tensor.matmul(
                        gqp[:, off * 9: off * 9 + 9],
                        xh1_sb[:, i * NTOK:(i + 1) * NTOK],
                        wfc_sp[:, e * 9:(e + 1) * 9],
                        start=False, stop=False)
                    nc.tensor.matmul(
                        gqp[:, off * 9: off * 9 + 9],
                        X[:, e * NTOK:(e + 1) * NTOK],
                        wfc_rem[:, e * 9:(e + 1) * 9],
                        start=False, stop=last)
            for g in range(ngrp):
                gqp, gn = qps_list[g]
                nc.scalar.activation(q_sb[:, g * 504: g * 504 + gn],
                                     gqp[:, :gn], AF.Copy)
            nc.sync.dma_start(out=q_t[:, :], in_=q_sb[:, :])
            nc.sync.dma_start(out=aux[:, :], in_=aux_sb[:, :])

    nc.finalize()
    return nc


# ----------------------------------------------------------------------------
# host entry
# ----------------------------------------------------------------------------

def _numpy_forward(ins):
    """Reference-equivalent numpy fallback (never used for sane inputs)."""
    x = _f32(ins["x"])
    b_sz = x.shape[0]
    W_row = _f32(ins["W_row"]).reshape(48, 90)
    W_col = _f32(ins["W_col"]).reshape(48, 90)
    W_box = _f32(ins["W_box"]).reshape(48, 90)

    def _ln(t, g, b):
        m = t.mean(-1, keepdims=True)
        v = t.var(-1, keepdims=True)
        return (t - m) / np.sqrt(v + 1e-5) * g + b

    row = np.einsum("fk,bkr->bfr",
                    W_row, x.transpose(0, 2, 3, 1).reshape(b_sz, 81, 10)
                    .reshape(b_sz, 9, 9, 10).transpose(0, 3, 1, 2)
                    .reshape(b_sz, 10, 9, 9).reshape(b_sz, 90, 9)) \
        if False else None
    # straightforward implementation:
    xr = x.reshape(b_sz, 10, 9, 9)
    row = np.einsum("bcrk,fck->bfr", xr, W_row.reshape(48, 10, 9)) \
        + _f32(ins["b_row"])[None, :, None]
    col = np.einsum("bckw,fck->bfw", xr, W_col.reshape(48, 10, 9)) \
        + _f32(ins["b_col"])[None, :, None]
    xb = xr.reshape(b_sz, 10, 3, 3, 3, 3)
    box = np.einsum("bcRiCj,fcij->bfRC", xb,
                    W_box.reshape(48, 10, 3, 3)) \
        + _f32(ins["b_box"])[None, :, None, None]
    W_red, b_red = _f32(ins["W_red"]), _f32(ins["b_red"])
    Wr, Wc, Wb = W_red[:, :48], W_red[:, 48:96], W_red[:, 96:144]
    A = np.einsum("of,bfr->bor", Wr, row)
    Cc = np.einsum("of,bfw->bow", Wc, col)
    Xx = np.einsum("of,bfRC->boRC", Wb, box)
    h = (A[:, :, :, None] + Cc[:, :, None, :]
         + np.repeat(np.repeat(Xx, 3, 2), 3, 3) + b_red[None, :, None, None])
    bn_g, bn_b = _f32(ins["bn_g"]), _f32(ins["bn_b"])
    bn_m, bn_v = _f32(ins["bn_m"]), _f32(ins["bn_v"])
    sc_ = bn_g / np.sqrt(bn_v + 1e-5)
    h = h * sc_[None, :, None, None] + (bn_b - bn_m * sc_)[None, :, None, None]
    h = np.maximum(h, 0.0)
    W_qkv, b_qkv = _f32(ins["W_qkv"]), _f32(ins["b_qkv"])
    W_o, b_o = _f32(ins["W_o"]), _f32(ins["b_o"])
    W_f1, b_f1 = _f32(ins["W_f1"]), _f32(ins["b_f1"])
    W_f2, b_f2 = _f32(ins["W_f2"]), _f32(ins["b_f2"])
    W_h, b_h = _f32(ins["W_h"]), _f32(ins["b_h"])
    W_fc, b_fc = _f32(ins["W_fc"]), _f32(ins["b_fc"])
    ln1_g, ln1_b = _f32(ins["ln1_g"]), _f32(ins["ln1_b"])
    ln2_g, ln2_b = _f32(ins["ln2_g"]), _f32(ins["ln2_b"])

    def reasoning(s):
        t = s.reshape(b_sz, D, 81).transpose(0, 2, 1)
        qkv = t @ W_qkv.T + b_qkv
        q, k, v = qkv[..., :D], qkv[..., D:2 * D], qkv[..., 2 * D:]
        q = q.reshape(b_sz, 81, H, DH)
        k = k.reshape(b_sz, 81, H, DH)
        v = v.reshape(b_sz, 81, H, DH)
        s_ = np.einsum("bqhd,bkhd->bhqk", q, k) / np.sqrt(np.float32(DH))
        s_ = s_ - s_.max(-1, keepdims=True)
        e = np.exp(s_)
        attn = e / e.sum(-1, keepdims=True)
        o = np.einsum("bhqk,bkhd->bqhd", attn, v).reshape(b_sz, 81, D)
        t = _ln(t + (o @ W_o.T + b_o), ln1_g, ln1_b)
        ffv = np.maximum(t @ W_f1.T + b_f1, 0.0) @ W_f2.T + b_f2
        t = _ln(t + ffv, ln2_g, ln2_b)
        ns = t.transpose(0, 2, 1).reshape(b_sz, D, 9, 9)
        gf = ns.mean(axis=(2, 3))
        hp = 1.0 / (1.0 + np.exp(-(gf @ W_h.T + b_h)))
        return ns, hp

    s = h
    halt_v = np.zeros((b_sz, 1), np.float32)
    ponder = np.zeros((b_sz,), np.float32)
    ssum = np.zeros_like(h)
    for _ in range(MAX_STEPS):
        if float(halt_v.max()) >= THRESH:
            break
        ns, hp = reasoning(s)
        sp = np.minimum(hp, 1.0 - halt_v)
        nh = halt_v + sp
        ponder = ponder + (nh < 1.0).astype(np.float32).squeeze(-1)
        ssum = ssum + ns * sp[:, :, None, None]
        s, halt_v = ns, nh
    rem = 1.0 - halt_v
    ssum = ssum + s * rem[:, :, None, None]
    ponder = ponder + rem.squeeze(-1)
    final = ssum.transpose(0, 2, 3, 1).reshape(b_sz, 81, D)
    q_values = (final @ W_fc.T + b_fc).reshape(b_sz, -1)
    return q_values.astype(np.float32), ponder.astype(np.float32)


PROFILE = False
_LAST_EXEC_NS = None
_LAST_RESULTS = None


def kernel(**inputs):
    from concourse.bass_utils import run_bass_kernel_spmd

    x = _f32(inputs["x"])
    b_full = x.shape[0]
    if b_full % NCORES != 0 or x.shape[1:] != (10, 9, 9):
        return _numpy_forward(inputs)
    bs = b_full // NCORES
    folded = _fold_weights(inputs)

    key = bs
    if key not in _BUILD_CACHE:
        _BUILD_CACHE[key] = _build(bs)
    nc = _BUILD_CACHE[key]

    xrow2d = np.ascontiguousarray(x.transpose(0, 2, 1, 3).reshape(b_full, 810))
    xcol2d = np.ascontiguousarray(x.transpose(0, 3, 1, 2).reshape(b_full, 810))
    xbox2d = np.ascontiguousarray(
        x.reshape(b_full, 10, 3, 3, 3, 3).transpose(0, 2, 4, 1, 3, 5)
        .reshape(b_full, 810))
    in_maps = []
    for c in range(NCORES):
        m = dict(folded)
        sl = slice(c * bs, (c + 1) * bs)
        m["xrow2d"] = xrow2d[sl]
        m["xcol2d"] = xcol2d[sl]
        m["xbox2d"] = xbox2d[sl]
        in_maps.append(m)
    global _LAST_EXEC_NS, _LAST_RESULTS
    import time as _time
    _t0 = _time.monotonic()
    r = run_bass_kernel_spmd(nc, in_maps, list(range(NCORES)), trace=PROFILE)
    _wall_ns = int((_time.monotonic() - _t0) * 1e9)
    _LAST_EXEC_NS = r.exec_time_ns if r.exec_time_ns is not None else _wall_ns
    _LAST_RESULTS = r
    res = r.results

    q_parts, p_parts = [], []
    hm1 = -1e30
    hm2 = -1e30
    for c in range(NCORES):
        qt = res[c]["q_t"]                      # (81, bs*9)
        au = res[c]["aux"][0]                   # (bs+8,)
        q_parts.append(qt.reshape(81, bs, 9).transpose(1, 0, 2).reshape(bs, 729))
        p_parts.append(au[:bs])
        hm1 = max(hm1, float(au[bs]))
        hm2 = max(hm2, float(au[bs + 1]))
    if not (hm1 < THRESH <= hm2):
        # Halting pattern differs from the fused 2-step fast path: recompute
        # exactly on host (slow, should never trigger for this model family).
        return _numpy_forward(inputs)
    q_values = np.concatenate(q_parts, axis=0)
    ponder = np.concatenate(p_parts, axis=0)
    return q_values, ponder


if __name__ == "__main__":
    d = np.load("/root/problem/inputs_cache.npz")
    ins = {k: d[k] for k in d.files}
    q, p = kernel(**ins)
    print("q", q.shape, "ponder", p.shape)


# revision 37
# speedup vs baseline: 1.5130x; 1.3453x over previous
"""Trainium2 Bass kernel for nn_DQNSolverCNN6 (Sudoku DQN with ACT halting).

Self-contained: host-side weight folding + 8-core SPMD bass kernel.

Structure (per core, batch shard BS=256, tokens T=BS*81):
  prologue: conv branches folded into 3 (128x90) matmuls + BN + ReLU -> s0
  2 transformer steps (global halting freezes the net after step 2 for any
  realistic input; host validates via halt-max outputs and falls back to a
  numpy path if the input behaves differently)
  epilogue: q = Wfc @ (sp1*ns1 + (1-sp1)*ns2) + bias, ponder from hp1/hp2.

Layouts: activations (128 d-partitions, T tokens) fp32 in SBUF; attention
per-element with (j, q) score layout; LayerNorm via PE-transpose land.
"""
import math
import numpy as np

B_FULL, D, H, DH = 2048, 128, 4, 32
NCORES = 8
BS = B_FULL // NCORES          # 256
NTOK = 81
MAX_STEPS, THRESH = 16, 0.99
CH = 4                         # elements per chunk (324 tokens)
SG = 2                         # elements per scores/exp sub-group

_BUILD_CACHE = {}


# ----------------------------------------------------------------------------
# host-side helpers
# ----------------------------------------------------------------------------

def _f32(x):
    return np.ascontiguousarray(np.asarray(x), dtype=np.float32)


def _fold_weights(ins):
    """Fold conv/BN/LN affines into matmul weights. Returns device tensors."""
    W_row = _f32(ins["W_row"]).reshape(48, 90)   # (f, c*9+k)
    W_col = _f32(ins["W_col"]).reshape(48, 90)   # (f, c*9+row)
    W_box = _f32(ins["W_box"]).reshape(48, 90)   # (f, c*9+i*3+j)
    b_row, b_col, b_box = _f32(ins["b_row"]), _f32(ins["b_col"]), _f32(ins["b_box"])
    W_red, b_red = _f32(ins["W_red"]), _f32(ins["b_red"])
    bn_g, bn_b = _f32(ins["bn_g"]), _f32(ins["bn_b"])
    bn_m, bn_v = _f32(ins["bn_m"]), _f32(ins["bn_v"])

    Wr, Wc, Wb = W_red[:, :48], W_red[:, 48:96], W_red[:, 96:144]
    bs_ = bn_g / np.sqrt(bn_v + 1e-5)
    M_row = (bs_[:, None] * (Wr @ W_row))            # (128, 90)
    M_col = (bs_[:, None] * (Wc @ W_col))
    M_box = (bs_[:, None] * (Wb @ W_box))
    cb = Wr @ b_row + Wc @ b_col + Wb @ b_box + b_red
    bias0 = bs_ * cb + (bn_b - bn_m * bs_)           # (128,)

    W_qkv, b_qkv = _f32(ins["W_qkv"]), _f32(ins["b_qkv"])
    W_o, b_o = _f32(ins["W_o"]), _f32(ins["b_o"])
    ln1_g, ln1_b = _f32(ins["ln1_g"]), _f32(ins["ln1_b"])
    W_f1, b_f1 = _f32(ins["W_f1"]), _f32(ins["b_f1"])
    W_f2, b_f2 = _f32(ins["W_f2"]), _f32(ins["b_f2"])
    ln2_g, ln2_b = _f32(ins["ln2_g"]), _f32(ins["ln2_b"])
    W_h, b_h = _f32(ins["W_h"]), _f32(ins["b_h"])
    W_fc, b_fc = _f32(ins["W_fc"]), _f32(ins["b_fc"])

    sc = 1.0 / math.sqrt(DH)
    Wq, Wk, Wv = W_qkv[:D], W_qkv[D:2 * D], W_qkv[2 * D:]
    bq, bk, bv = b_qkv[:D], b_qkv[D:2 * D], b_qkv[2 * D:]

    out = {
        "mrowT": M_row.T.copy(), "mcolT": M_col.T.copy(), "mboxT": M_box.T.copy(),
        "bias0_row": bias0.reshape(1, 128).copy(),
    }
    # step 1 input is real s0; step 2 input is x-hat with affine (ln2_g, ln2_b)
    for s, (g_in, b_in) in (("1", (None, None)), ("2", (ln2_g, ln2_b))):
        def fold(W, b):
            if g_in is None:
                return W.copy(), b.copy()
            return W * g_in[None, :], b + W @ b_in
        Wq_s, bq_s = fold(Wq * sc, bq * sc)
        Wk_s, bk_s = fold(Wk, bk)
        Wv_s, bv_s = fold(Wv, bv)
        out[f"wqT_{s}"] = Wq_s.T.copy()
        out[f"wkT_{s}"] = Wk_s.T.copy()
        out[f"wvT_{s}"] = Wv_s.T.copy()
        out[f"bq_row_{s}"] = bq_s.reshape(1, 128).copy()
        out[f"bk_row_{s}"] = bk_s.reshape(1, 128).copy()
        out[f"bo_row_{s}"] = (b_o + W_o @ bv_s).reshape(1, 128).copy()
        out[f"diag_{s}"] = (np.eye(D, dtype=np.float32) if g_in is None
                            else np.diag(g_in).astype(np.float32))
    out["woT"] = W_o.T.copy()
    # ffn with ln1 fold (same both steps)
    W_f1p = W_f1 * ln1_g[None, :]
    b_f1p = b_f1 + W_f1 @ ln1_b
    out["wf1T"] = W_f1p.T.copy()                  # (128, 512)
    out["bf1_cols"] = b_f1p.reshape(4, 128).T.copy()  # (128, 4) col f = bias of tile f
    wf2t = W_f2.T.copy()                          # (512, 128)
    out["wf2T"] = np.concatenate([wf2t[k * 128:(k + 1) * 128, :] for k in range(4)],
                                 axis=1)          # (128, 4*128), block k = slice k
    out["diag_g1"] = np.diag(ln1_g).astype(np.float32)
    out["bw_row"] = (b_f2 + ln1_b).reshape(1, 128).copy()
    # halting path (ln2 fold, same both steps)
    out["wh_col"] = ((W_h[0] * ln2_g) / 81.0).reshape(128, 1).copy()
    out["neg_bh"] = np.array([[-(float(W_h[0] @ ln2_b) + float(b_h[0]))]],
                             dtype=np.float32)
    # epilogue
    Wfcp = W_fc * ln2_g[None, :]
    bfcp = b_fc + W_fc @ ln2_b
    out["wfcT"] = Wfcp.T.copy()                   # (128, 9)
    out["bfc_row"] = bfcp.reshape(1, 9).copy()
    # device constants (host-supplied so matmul operands have one DMA producer)
    out["ident"] = np.eye(128, dtype=np.float32)
    mh = np.zeros((81, 512), dtype=np.float32)
    for h in range(H):
        mh[:, h * 128 + 32 * h: h * 128 + 32 * h + 32] = 1.0
    out["maskh"] = mh
    return out


# ----------------------------------------------------------------------------
# bass program
# ----------------------------------------------------------------------------

def _build(bs):
    import concourse.bass as bass
    import concourse.bacc as bacc
    import concourse.tile as tile
    from concourse import mybir
    from contextlib import ExitStack

    FP32 = mybir.dt.float32
    FP32R = mybir.dt.float32r
    AF = mybir.ActivationFunctionType
    ALU = mybir.AluOpType

    T = bs * NTOK
    n_chunk = (bs + CH - 1) // CH

    def r_(ap):
        # fp32r is rejected by the walrus verifier on this toolchain;
        # keep the hook so a faster dtype can be swapped in later.
        return ap

    nc = bacc.Bacc()

    def param(name, shape):
        return nc.declare_dram_parameter(name, list(shape), FP32, isOutput=False)

    xrow2d = param("xrow2d", (bs, 810))
    xcol2d = param("xcol2d", (bs, 810))
    xbox2d = param("xbox2d", (bs, 810))
    names_128 = ["wqT_1", "wkT_1", "wvT_1", "wqT_2", "wkT_2", "wvT_2",
                 "woT", "diag_1", "diag_2", "diag_g1"]
    P = {n: param(n, (128, 128)) for n in names_128}
    P["wf1T"] = param("wf1T", (128, 512))
    P["wf2T"] = param("wf2T", (128, 512))
    P["mrowT"] = param("mrowT", (90, 128))
    P["mcolT"] = param("mcolT", (90, 128))
    P["mboxT"] = param("mboxT", (90, 128))
    P["bf1_cols"] = param("bf1_cols", (128, 4))
    P["wh_col"] = param("wh_col", (128, 1))
    P["wfcT"] = param("wfcT", (128, 9))
    rows = ["bias0_row", "bq_row_1", "bk_row_1", "bq_row_2", "bk_row_2",
            "bo_row_1", "bo_row_2", "bw_row"]
    for n in rows:
        P[n] = param(n, (1, 128))
    P["bfc_row"] = param("bfc_row", (1, 9))
    P["neg_bh"] = param("neg_bh", (1, 1))
    P["ident"] = param("ident", (128, 128))
    P["maskh"] = param("maskh", (81, 512))

    q_t = nc.declare_dram_parameter("q_t", [81, bs * 9], FP32, isOutput=True)
    aux = nc.declare_dram_parameter("aux", [1, bs + 8], FP32, isOutput=True)
    xh1_dram = nc.dram_tensor("xh1_spill", [128, T], FP32)

    def fv(t, dims, off=0):
        """free-dim view of an AP: keep partition dim, replace free dims."""
        return bass.AP(tensor=t.tensor, offset=t.offset + off,
                       ap=[t.ap[0]] + [list(d) for d in dims])

    with ExitStack() as ctx:
        tc = ctx.enter_context(tile.TileContext(nc))
        const = ctx.enter_context(tc.tile_pool(name="const", bufs=1))
        state = ctx.enter_context(tc.tile_pool(name="state", bufs=1))
        work = ctx.enter_context(tc.tile_pool(name="work", bufs=2))
        halt = ctx.enter_context(tc.tile_pool(name="halt", bufs=1))

        # ---- constants into SBUF ----
        W = {}
        for n, p in P.items():
            shp = list(p.shape)
            tl = const.tile(shp, FP32, tag=f"w_{n}")
            nc.sync.dma_start(out=tl[:, :], in_=p[:, :])
            W[n] = tl
        ident = W["ident"]
        maskh = W["maskh"]
        ones_row = const.tile([1, 512], FP32, tag="ones_row")
        nc.vector.memset(ones_row, 1.0)
        eps_col = const.tile([128, 1], FP32, tag="eps")
        nc.vector.memset(eps_col, 1e-5)

        def transpose(out_ap, in_ap):
            # Bacc.generate_event_semaphores splits multi-waits, so the
            # pe_touch wait-absorber is no longer needed per transpose.
            p = in_ap.partition_size()
            nc.tensor.transpose(out_ap, in_ap, ident[:p, :p])

        X = state.tile([128, T], FP32, tag="X")          # the state buffer
        q_sb = state.tile([81, bs * 9], FP32, tag="q_sb")
        aux_sb = halt.tile([1, bs + 8], FP32, tag="aux")
        nc.vector.memset(aux_sb, 0.0)

        # ================= prologue =================
        with tc.tile_pool(name="pro_ps", bufs=2, space="PSUM") as pro_ps, \
             tc.tile_pool(name="pro_sb", bufs=2) as pro_sb:
            # load the three host-rearranged x layouts; transposes then read
            # contiguous (hb, 90) slices (walrus: 1 free dim on PE operands).
            nhalf = (bs + 127) // 128
            Xrow = pro_sb.tile([90, bs * 9], FP32, tag="Xrow", bufs=1)
            Xcol = pro_sb.tile([90, bs * 9], FP32, tag="Xcol", bufs=1)
            Xbox = pro_sb.tile([90, bs * 9], FP32, tag="Xbox", bufs=1)
            for i in range(nhalf):
                hb = min(128, bs - i * 128)
                xr = pro_sb.tile([128, 810], FP32, tag="xr_sb")
                xc = pro_sb.tile([128, 810], FP32, tag="xc_sb")
                xb = pro_sb.tile([128, 810], FP32, tag="xb_sb")
                nc.sync.dma_start(out=xr[:hb, :], in_=xrow2d[i * 128:i * 128 + hb, :])
                nc.sync.dma_start(out=xc[:hb, :], in_=xcol2d[i * 128:i * 128 + hb, :])
                nc.sync.dma_start(out=xb[:hb, :], in_=xbox2d[i * 128:i * 128 + hb, :])
                for r in range(9):
                    pt = pro_ps.tile([90, 512], FP32, tag="tp")
                    transpose(pt[:, :hb], xr[:hb, r * 90:(r + 1) * 90])
                    nc.vector.tensor_copy(
                        fv(Xrow[:, :], [[9, hb]], off=i * 128 * 9 + r), pt[:, :hb])
                for w_ in range(9):
                    pt = pro_ps.tile([90, 512], FP32, tag="tp")
                    transpose(pt[:, :hb], xc[:hb, w_ * 90:(w_ + 1) * 90])
                    nc.vector.tensor_copy(
                        fv(Xcol[:, :], [[9, hb]], off=i * 128 * 9 + w_), pt[:, :hb])
                for rc in range(9):
                    pt = pro_ps.tile([90, 512], FP32, tag="tp")
                    transpose(pt[:, :hb], xb[:hb, rc * 90:(rc + 1) * 90])
                    nc.vector.tensor_copy(
                        fv(Xbox[:, :], [[9, hb]], off=i * 128 * 9 + rc), pt[:, :hb])
            # A-branch matmuls on the compact (b, pos) tensors, then the
            # 9x broadcasts happen as small elementwise adds per chunk.
            Arow = pro_sb.tile([128, bs * 9], FP32, tag="Arow", bufs=1)
            Acol = pro_sb.tile([128, bs * 9], FP32, tag="Acol", bufs=1)
            Abox = pro_sb.tile([128, bs * 9], FP32, tag="Abox", bufs=1)
            nbr = bs * 9
            for o0 in range(0, nbr, 512):
                on = min(512, nbr - o0)
                for nm, src_t, dst in (("mrowT", Xrow, Arow), ("mcolT", Xcol, Acol),
                                       ("mboxT", Xbox, Abox)):
                    ps_ = pro_ps.tile([128, 512], FP32, tag="aps", bufs=2)
                    if nm == "mrowT":
                        nc.tensor.matmul(r_(ps_[:, :on]), r_(W["bias0_row"][:, :]),
                                         r_(ones_row[:1, :on]),
                                         start=True, stop=False)
                    nc.tensor.matmul(r_(ps_[:, :on]), r_(W[nm][:, :]),
                                     src_t[:, o0:o0 + on],
                                     start=(nm != "mrowT"), stop=True)
                    nc.scalar.activation(dst[:, o0:o0 + on], ps_[:, :on], AF.Copy)
            # combine + relu per chunk
            for c in range(n_chunk):
                e0 = c * CH
                ce = min(CH, bs - e0)
                tw = ce * NTOK
                pre = pro_sb.tile([128, CH * NTOK], FP32, tag="pre", bufs=1)
                # pre[(e,r,c)] = Arow[(e,r)] + Acol[(e,c)]
                nc.vector.tensor_tensor(
                    pre[:, :tw],
                    fv(Arow[:, :], [[9, ce], [1, 9], [0, 9]], off=e0 * 9),
                    fv(Acol[:, :], [[9, ce], [0, 9], [1, 9]], off=e0 * 9),
                    ALU.add)
                pre2 = pro_sb.tile([128, CH * NTOK], FP32, tag="pre2", bufs=1)
                boxj = pro_sb.tile([128, CH * 27], FP32, tag="boxj", bufs=1)
                nc.gpsimd.tensor_copy(
                    boxj[:, :ce * 27],
                    fv(Abox[:, :], [[9, ce], [1, 9], [0, 3]], off=e0 * 9))
                for i in range(3):
                    nc.vector.tensor_tensor(
                        fv(pre2[:, :], [[81, ce], [27, 3], [1, 9]], off=9 * i),
                        fv(pre[:, :], [[81, ce], [27, 3], [1, 9]], off=9 * i),
                        fv(boxj[:, :], [[27, ce], [9, 3], [1, 9]]),
                        ALU.add)
                nc.scalar.activation(X[:, e0 * NTOK:e0 * NTOK + tw],
                                     pre2[:, :tw], AF.Relu)

        # ================= transformer steps =================
        sp1_row = halt.tile([1, bs], FP32, tag="sp1")
        rsp1_row = halt.tile([1, bs], FP32, tag="rsp1")

        def halting(step):
            """gf -> logit -> hp; update halting state. X holds x-hat."""
            gfs = halt.tile([128, bs], FP32, tag="gfs")
            nc.vector.tensor_reduce(gfs[:, :], fv(X[:, :], [[81, bs], [1, 81]]),
                                    axis=mybir.AxisListType.X, op=ALU.add)
            with tc.tile_pool(name=f"lg_ps{step}", bufs=1, space="PSUM") as lgp:
                lg = lgp.tile([1, 512 * ((bs + 511) // 512)], FP32, tag="lg")
                nc.tensor.matmul(lg[:1, :bs], W["wh_col"][:, :], gfs[:, :],
                                 start=True, stop=True)
                epx = halt.tile([1, bs], FP32, tag="epx")
                nc.scalar.activation(epx[:, :], lg[:1, :bs], AF.Exp,
                                     bias=W["neg_bh"][:1, :1], scale=-1.0)
            hp = halt.tile([1, bs], FP32, tag=f"hp{step}")
            nc.vector.tensor_scalar(hp[:, :], epx[:, :], 1.0, None, ALU.add)
            nc.vector.reciprocal(hp[:, :], hp[:, :])
            if step == 1:
                nc.vector.tensor_copy(sp1_row[:, :], hp[:, :])
                nc.vector.tensor_scalar(rsp1_row[:, :], hp[:, :], -1.0, 1.0,
                                        ALU.mult, ALU.add)
                nc.vector.tensor_reduce(aux_sb[:, bs:bs + 1], hp[:, :],
                                        axis=mybir.AxisListType.X, op=ALU.max)
            else:
                sp2 = halt.tile([1, bs], FP32, tag="sp2")
                nc.vector.tensor_tensor(sp2[:, :], hp[:, :], rsp1_row[:, :],
                                        ALU.min)
                nh2 = halt.tile([1, bs], FP32, tag="nh2")
                nc.vector.tensor_tensor(nh2[:, :], sp1_row[:, :], sp2[:, :],
                                        ALU.add)
                nc.vector.tensor_reduce(aux_sb[:, bs + 1:bs + 2], nh2[:, :],
                                        axis=mybir.AxisListType.X, op=ALU.max)
                rem = halt.tile([1, bs], FP32, tag="rem")
                nc.vector.tensor_scalar(rem[:, :], nh2[:, :], -1.0, 1.0,
                                        ALU.mult, ALU.add)
                ind = halt.tile([1, bs], FP32, tag="ind")
                nc.scalar.sign(ind[:, :], rem[:, :])
                nc.vector.tensor_scalar(ind[:, :], ind[:, :], 0.0, None, ALU.max)
                pond = halt.tile([1, bs], FP32, tag="pond")
                nc.vector.tensor_tensor(pond[:, :], ind[:, :], rem[:, :], ALU.add)
                nc.vector.tensor_scalar(aux_sb[:, 0:bs], pond[:, :], 1.0, None,
                                        ALU.add)

        def attention_block(step):
            s = str(step)
            with tc.tile_pool(name=f"a_ps{step}", bufs=1, space="PSUM") as ps1, \
                 tc.tile_pool(name=f"a_ps2{step}", bufs=2, space="PSUM") as ps2, \
                 tc.tile_pool(name=f"a_sb{step}", bufs=2) as asb:
                for c in range(n_chunk):
                    e0 = c * CH
                    ce = min(CH, bs - e0)
                    tw = ce * NTOK
                    t0 = e0 * NTOK
                    Xc = X[:, t0:t0 + tw]
                    # --- q, k ---
                    qps = ps2.tile([128, 512], FP32, tag="qk")
                    kps = ps2.tile([128, 512], FP32, tag="qk")
                    nc.tensor.matmul(r_(qps[:, :tw]), r_(W[f"bq_row_{s}"][:, :]),
                                     r_(ones_row[:1, :tw]), start=True, stop=False)
                    nc.tensor.matmul(r_(qps[:, :tw]), r_(W[f"wqT_{s}"][:, :]),
                                     r_(Xc), start=False, stop=True)
                    nc.tensor.matmul(r_(kps[:, :tw]), r_(W[f"bk_row_{s}"][:, :]),
                                     r_(ones_row[:1, :tw]), start=True, stop=False)
                    nc.tensor.matmul(r_(kps[:, :tw]), r_(W[f"wkT_{s}"][:, :]),
                                     r_(Xc), start=False, stop=True)
                    q_c = asb.tile([128, CH * NTOK], FP32, tag="q_c")
                    k_c = asb.tile([128, CH * NTOK], FP32, tag="k_c")
                    nc.vector.tensor_copy(q_c[:, :tw], qps[:, :tw])
                    nc.scalar.activation(k_c[:, :tw], kps[:, :tw], AF.Copy)
                    # --- v (token-partition layout) ---
                    vps = ps1.tile([81, 512], FP32, tag="acc", bufs=2)
                    for i in range(ce):
                        nc.tensor.matmul(
                            vps[:, i * 128:(i + 1) * 128],
                            X[:, t0 + i * NTOK: t0 + (i + 1) * NTOK],
                            W[f"wvT_{s}"][:, :], start=True, stop=True)
                    v_c = asb.tile([81, CH * 128], FP32, tag="v_c")
                    nc.scalar.activation(v_c[:, :ce * 128], vps[:, :ce * 128],
                                         AF.Copy)
                    # --- scores + exp; psum/e_c laid out (h, e, q) so the
                    # denb and attnv matmul rhs operands are 1-D slices ---
                    e_c = asb.tile([81, CH * 324], FP32, tag="e_c")
                    scp = ps1.tile([81, 2048], FP32, tag="scp", bufs=1)
                    for i in range(ce):
                        for h in range(H):
                            nc.tensor.matmul(
                                scp[:81, h * 512 + i * 81: h * 512 + i * 81 + 81],
                                k_c[32 * h:32 * h + 32, i * NTOK:(i + 1) * NTOK],
                                q_c[32 * h:32 * h + 32, i * NTOK:(i + 1) * NTOK],
                                start=True, stop=True,
                                tile_position=(32 * h, 0))
                    nc.scalar.activation(
                        fv(e_c[:, :], [[ce * 81, 4], [1, ce * 81]]),
                        fv(scp[:, :], [[512, 4], [1, ce * 81]]),
                        AF.Exp)
                    # --- denominators broadcast ---
                    dps = ps1.tile([128, 512], FP32, tag="acc", bufs=2)
                    for h in range(H):
                        nc.tensor.matmul(
                            r_(dps[:, :tw]), r_(maskh[:, h * 128:(h + 1) * 128]),
                            e_c[:81, h * ce * 81:(h + 1) * ce * 81],
                            start=(h == 0), stop=(h == H - 1))
                    rdb = asb.tile([128, CH * NTOK], FP32, tag="rdb")
                    nc.vector.reciprocal(rdb[:, :tw], dps[:, :tw])
                    # --- attn @ v (col-packed by head) ---
                    ops_ = ps1.tile([128, 512], FP32, tag="acc", bufs=2)
                    for i in range(ce):
                        for h in range(H):
                            nc.tensor.matmul(
                                ops_[32 * h:32 * h + 32, i * NTOK:(i + 1) * NTOK],
                                v_c[:81, i * 128 + 32 * h: i * 128 + 32 * h + 32],
                                e_c[:81, h * ce * 81 + i * 81: h * ce * 81 + (i + 1) * 81],
                                start=True, stop=True,
                                tile_position=(0, 32 * h))
                    o_c = asb.tile([128, CH * NTOK], FP32, tag="o_c")
                    nc.vector.tensor_tensor(o_c[:, :tw], ops_[:, :tw],
                                            rdb[:, :tw], ALU.mult)
                    # --- Wo + residual + bias ---
                    ups = ps1.tile([128, 512], FP32, tag="acc", bufs=2)
                    nc.tensor.matmul(r_(ups[:, :tw]), r_(W[f"bo_row_{s}"][:, :]),
                                     r_(ones_row[:1, :tw]), start=True, stop=False)
                    nc.tensor.matmul(r_(ups[:, :tw]), r_(W["woT"][:, :]),
                                     r_(o_c[:, :tw]), start=False, stop=False)
                    nc.tensor.matmul(r_(ups[:, :tw]), r_(W[f"diag_{s}"][:, :]),
                                     r_(Xc), start=False, stop=True)
                    nc.scalar.activation(Xc, ups[:, :tw], AF.Copy)

        def layernorm_block(step, tag):
            ntile = (T + 127) // 128
            SC = 4
            with tc.tile_pool(name=f"ln_ps_{tag}{step}", bufs=3, space="PSUM") as lps, \
                 tc.tile_pool(name=f"ln_sb_{tag}{step}", bufs=2) as lsb:
                for sc0 in range(0, ntile, SC):
                    scn = min(SC, ntile - sc0)
                    fwd = lps.tile([128, SC * 128], FP32, tag="fwd")
                    bwd = lps.tile([128, SC * 128], FP32, tag="bwd")
                    mv = lsb.tile([128, SC * 2], FP32, tag="mv")
                    rr = lsb.tile([128, SC], FP32, tag="rr")
                    nc.vector.memset(mv, 1.0)
                    sizes = []
                    for ti in range(scn):
                        t_ = sc0 + ti
                        w_ = min(128, T - t_ * 128)
                        sizes.append(w_)
                        transpose(fwd[:w_, ti * 128: ti * 128 + 128],
                                  X[:, t_ * 128: t_ * 128 + w_])
                        st6 = lsb.tile([128, 6], FP32, tag="st6")
                        nc.vector.bn_stats(st6[:w_, :],
                                           fwd[:w_, ti * 128: ti * 128 + 128])
                        nc.vector.bn_aggr(mv[:w_, ti * 2: ti * 2 + 2], st6[:w_, :])
                    wmax = max(sizes)
                    nc.scalar.activation(rr[:wmax, :scn],
                                         fv(mv[:wmax, :], [[2, scn]], off=1),
                                         AF.Ln, bias=eps_col[:wmax, :])
                    nc.scalar.activation(rr[:wmax, :scn], rr[:wmax, :scn],
                                         AF.Exp, scale=-0.5)
                    for ti in range(scn):
                        t_ = sc0 + ti
                        w_ = sizes[ti]
                        xh_t = lsb.tile([128, 128], FP32, tag="xh_t")
                        nc.vector.tensor_scalar(
                            xh_t[:w_, :], fwd[:w_, ti * 128: ti * 128 + 128],
                            mv[:w_, ti * 2: ti * 2 + 1],
                            rr[:w_, ti: ti + 1],
                            ALU.subtract, ALU.mult)
                        transpose(bwd[:, ti * 128: ti * 128 + w_],
                                  xh_t[:w_, :])
                    for ti in range(scn):
                        t_ = sc0 + ti
                        w_ = sizes[ti]
                        nc.scalar.activation(X[:, t_ * 128: t_ * 128 + w_],
                                             bwd[:, ti * 128: ti * 128 + w_],
                                             AF.Copy)

        def ffn_block(step):
            with tc.tile_pool(name=f"f_ps{step}", bufs=2, space="PSUM") as fps, \
                 tc.tile_pool(name=f"f_ps2{step}", bufs=4, space="PSUM") as fps2, \
                 tc.tile_pool(name=f"f_sb{step}", bufs=2) as fsb:
                for c in range(n_chunk):
                    e0 = c * CH
                    ce = min(CH, bs - e0)
                    tw = ce * NTOK
                    t0 = e0 * NTOK
                    Xc = X[:, t0:t0 + tw]
                    ff = fsb.tile([128, 4 * CH * NTOK], FP32, tag="ff")
                    for f in range(4):
                        fp = fps2.tile([128, 512], FP32, tag="fp")
                        nc.tensor.matmul(r_(fp[:, :tw]),
                                         r_(W["wf1T"][:, f * 128:(f + 1) * 128]),
                                         r_(Xc), start=True, stop=True)
                        nc.scalar.activation(
                            ff[:, f * tw: (f + 1) * tw], fp[:, :tw], AF.Relu,
                            bias=W["bf1_cols"][:, f:f + 1])
                    wp = fps.tile([128, 512], FP32, tag="wp")
                    nc.tensor.matmul(r_(wp[:, :tw]), r_(W["bw_row"][:, :]),
                                     r_(ones_row[:1, :tw]), start=True, stop=False)
                    for k in range(4):
                        nc.tensor.matmul(r_(wp[:, :tw]),
                                         r_(W["wf2T"][:, k * 128:(k + 1) * 128]),
                                         r_(ff[:, k * tw:(k + 1) * tw]),
                                         start=False, stop=False)
                    nc.tensor.matmul(r_(wp[:, :tw]), r_(W["diag_g1"][:, :]),
                                     r_(Xc), start=False, stop=True)
                    nc.scalar.activation(Xc, wp[:, :tw], AF.Copy)

        # ---- step 1 ----
        attention_block(1)
        layernorm_block(1, "a")
        ffn_block(1)
        layernorm_block(1, "b")
        halting(1)
        nc.sync.dma_start(out=xh1_dram[:, :], in_=X[:, :])

        # ---- step 2 ----
        attention_block(2)
        layernorm_block(2, "a")
        ffn_block(2)
        layernorm_block(2, "b")
        halting(2)

        # ================= epilogue =================
        with tc.tile_pool(name="e_ps", bufs=2, space="PSUM") as eps_ps, \
             tc.tile_pool(name="e_qp", bufs=1, space="PSUM") as eqp_ps, \
             tc.tile_pool(name="e_sb", bufs=2) as esb:
            wfc_sp = esb.tile([128, bs * 9], FP32, tag="wfc_sp", bufs=1)
            wfc_rem = esb.tile([128, bs * 9], FP32, tag="wfc_rem", bufs=1)
            sp1_9 = esb.tile([1, bs * 9], FP32, tag="sp1_9", bufs=1)
            nc.vector.tensor_copy(sp1_9[:1, :],
                                  fv(sp1_row[:1, :], [[1, bs], [0, 9]]))
            nw = bs * 9
            for o0 in range(0, nw, 504):
                on = min(504, nw - o0)
                e_lo = o0 // 9
                bc = eps_ps.tile([128, 512], FP32, tag="bc")
                nc.tensor.matmul(
                    r_(bc[:, :on]), r_(ones_row[:1, :128]),
                    sp1_9[:1, o0:o0 + on],
                    start=True, stop=True)
                nc.vector.tensor_tensor(
                    wfc_sp[:, o0:o0 + on],
                    fv(W["wfcT"][:, :], [[0, (on + 8) // 9], [1, 9]]),
                    bc[:, :on], ALU.mult)
            nc.gpsimd.tensor_tensor(
                wfc_rem[:, :], fv(W["wfcT"][:, :], [[0, bs], [1, 9]]),
                wfc_sp[:, :], ALU.subtract)
            # q-psums: 81 x 504 banks
            xh1_sb = None
            ngrp = (bs + 55) // 56
            qps_list = []
            bfc_rep = esb.tile([1, 504], FP32, tag="bfc_rep", bufs=1)
            nc.vector.tensor_copy(bfc_rep[:1, :],
                                  fv(W["bfc_row"][:1, :], [[0, 56], [1, 9]]))
            for g in range(ngrp):
                gqp = eqp_ps.tile([81, 512], FP32, tag=f"qp{g}")
                gn = min(56, bs - g * 56) * 9
                nc.tensor.matmul(
                    r_(gqp[:, :gn]), r_(ones_row[:1, :81]),
                    bfc_rep[:1, :gn],
                    start=True, stop=False)
                qps_list.append((gqp, gn))
            for c in range(n_chunk):
                e0 = c * CH
                ce = min(CH, bs - e0)
                tw = ce * NTOK
                xh1_sb = esb.tile([128, CH * NTOK], FP32, tag="xh1_sb")
                nc.sync.dma_start(out=xh1_sb[:, :tw],
                                  in_=xh1_dram[:, e0 * NTOK: e0 * NTOK + tw])
                for i in range(ce):
                    e = e0 + i
                    g, off = divmod(e, 56)
                    gqp, gn = qps_list[g]
                    last = e == bs - 1 or off == 55
                    nc.tensor.matmul(
                        gqp[:, off * 9: off * 9 + 9],
                        xh1_sb[:, i * NTOK:(i + 1) * NTOK],
                        wfc_sp[:, e * 9:(e + 1) * 9],
                        start=False, stop=False)
                    nc.tensor.matmul(
                        gqp[:, off * 9: off * 9 + 9],
                        X[:, e * NTOK:(e + 1) * NTOK],
                        wfc_rem[:, e * 9:(e + 1) * 9],
                        start=False, stop=last)
            for g in range(ngrp):
                gqp, gn = qps_list[g]
                nc.scalar.activation(q_sb[:, g * 504: g * 504 + gn],
                                     gqp[:, :gn], AF.Copy)
            nc.sync.dma_start(out=q_t[:, :], in_=q_sb[:, :])
            nc.sync.dma_start(out=aux[:, :], in_=aux_sb[:, :])

    nc.finalize()
    return nc


# ----------------------------------------------------------------------------
# host entry
# ----------------------------------------------------------------------------

def _numpy_forward(ins):
    """Reference-equivalent numpy fallback (never used for sane inputs)."""
    x = _f32(ins["x"])
    b_sz = x.shape[0]
    W_row = _f32(ins["W_row"]).reshape(48, 90)
    W_col = _f32(ins["W_col"]).reshape(48, 90)
    W_box = _f32(ins["W_box"]).reshape(48, 90)

    def _ln(t, g, b):
        m = t.mean(-1, keepdims=True)
        v = t.var(-1, keepdims=True)
        return (t - m) / np.sqrt(v + 1e-5) * g + b

    row = np.einsum("fk,bkr->bfr",
                    W_row, x.transpose(0, 2, 3, 1).reshape(b_sz, 81, 10)
                    .reshape(b_sz, 9, 9, 10).transpose(0, 3, 1, 2)
                    .reshape(b_sz, 10, 9, 9).reshape(b_sz, 90, 9)) \
        if False else None
    # straightforward implementation:
    xr = x.reshape(b_sz, 10, 9, 9)
    row = np.einsum("bcrk,fck->bfr", xr, W_row.reshape(48, 10, 9)) \
        + _f32(ins["b_row"])[None, :, None]
    col = np.einsum("bckw,fck->bfw", xr, W_col.reshape(48, 10, 9)) \
        + _f32(ins["b_col"])[None, :, None]
    xb = xr.reshape(b_sz, 10, 3, 3, 3, 3)
    box = np.einsum("bcRiCj,fcij->bfRC", xb,
                    W_box.reshape(48, 10, 3, 3)) \
        + _f32(ins["b_box"])[None, :, None, None]
    W_red, b_red = _f32(ins["W_red"]), _f32(ins["b_red"])
    Wr, Wc, Wb = W_red[:, :48], W_red[:, 48:96], W_red[:, 96:144]
    A = np.einsum("of,bfr->bor", Wr, row)
    Cc = np.einsum("of,bfw->bow", Wc, col)
    Xx = np.einsum("of,bfRC->boRC", Wb, box)
    h = (A[:, :, :, None] + Cc[:, :, None, :]
         + np.repeat(np.repeat(Xx, 3, 2), 3, 3) + b_red[None, :, None, None])
    bn_g, bn_b = _f32(ins["bn_g"]), _f32(ins["bn_b"])
    bn_m, bn_v = _f32(ins["bn_m"]), _f32(ins["bn_v"])
    sc_ = bn_g / np.sqrt(bn_v + 1e-5)
    h = h * sc_[None, :, None, None] + (bn_b - bn_m * sc_)[None, :, None, None]
    h = np.maximum(h, 0.0)
    W_qkv, b_qkv = _f32(ins["W_qkv"]), _f32(ins["b_qkv"])
    W_o, b_o = _f32(ins["W_o"]), _f32(ins["b_o"])
    W_f1, b_f1 = _f32(ins["W_f1"]), _f32(ins["b_f1"])
    W_f2, b_f2 = _f32(ins["W_f2"]), _f32(ins["b_f2"])
    W_h, b_h = _f32(ins["W_h"]), _f32(ins["b_h"])
    W_fc, b_fc = _f32(ins["W_fc"]), _f32(ins["b_fc"])
    ln1_g, ln1_b = _f32(ins["ln1_g"]), _f32(ins["ln1_b"])
    ln2_g, ln2_b = _f32(ins["ln2_g"]), _f32(ins["ln2_b"])

    def reasoning(s):
        t = s.reshape(b_sz, D, 81).transpose(0, 2, 1)
        qkv = t @ W_qkv.T + b_qkv
        q, k, v = qkv[..., :D], qkv[..., D:2 * D], qkv[..., 2 * D:]
        q = q.reshape(b_sz, 81, H, DH)
        k = k.reshape(b_sz, 81, H, DH)
        v = v.reshape(b_sz, 81, H, DH)
        s_ = np.einsum("bqhd,bkhd->bhqk", q, k) / np.sqrt(np.float32(DH))
        s_ = s_ - s_.max(-1, keepdims=True)
        e = np.exp(s_)
        attn = e / e.sum(-1, keepdims=True)
        o = np.einsum("bhqk,bkhd->bqhd", attn, v).reshape(b_sz, 81, D)
        t = _ln(t + (o @ W_o.T + b_o), ln1_g, ln1_b)
        ffv = np.maximum(t @ W_f1.T + b_f1, 0.0) @ W_f2.T + b_f2
        t = _ln(t + ffv, ln2_g, ln2_b)
        ns = t.transpose(0, 2, 1).reshape(b_sz, D, 9, 9)
        gf = ns.mean(axis=(2, 3))
        hp = 1.0 / (1.0 + np.exp(-(gf @ W_h.T + b_h)))
        return ns, hp

    s = h
    halt_v = np.zeros((b_sz, 1), np.float32)
    ponder = np.zeros((b_sz,), np.float32)
    ssum = np.zeros_like(h)
    for _ in range(MAX_STEPS):
        if float(halt_v.max()) >= THRESH:
            break
        ns, hp = reasoning(s)
        sp = np.minimum(hp, 1.0 - halt_v)
        nh = halt_v + sp
        ponder = ponder + (nh < 1.0).astype(np.float32).squeeze(-1)
        ssum = ssum + ns * sp[:, :, None, None]
        s, halt_v = ns, nh
    rem = 1.0 - halt_v
    ssum = ssum + s * rem[:, :, None, None]
    ponder = ponder + rem.squeeze(-1)
    final = ssum.transpose(0, 2, 3, 1).reshape(b_sz, 81, D)
    q_values = (final @ W_fc.T + b_fc).reshape(b_sz, -1)
    return q_values.astype(np.float32), ponder.astype(np.float32)


PROFILE = False
_LAST_EXEC_NS = None
_LAST_RESULTS = None


def kernel(**inputs):
    from concourse.bass_utils import run_bass_kernel_spmd

    x = _f32(inputs["x"])
    b_full = x.shape[0]
    if b_full % NCORES != 0 or x.shape[1:] != (10, 9, 9):
        return _numpy_forward(inputs)
    bs = b_full // NCORES
    folded = _fold_weights(inputs)

    key = bs
    if key not in _BUILD_CACHE:
        _BUILD_CACHE[key] = _build(bs)
    nc = _BUILD_CACHE[key]

    xrow2d = np.ascontiguousarray(x.transpose(0, 2, 1, 3).reshape(b_full, 810))
    xcol2d = np.ascontiguousarray(x.transpose(0, 3, 1, 2).reshape(b_full, 810))
    xbox2d = np.ascontiguousarray(
        x.reshape(b_full, 10, 3, 3, 3, 3).transpose(0, 2, 4, 1, 3, 5)
        .reshape(b_full, 810))
    in_maps = []
    for c in range(NCORES):
        m = dict(folded)
        sl = slice(c * bs, (c + 1) * bs)
        m["xrow2d"] = xrow2d[sl]
        m["xcol2d"] = xcol2d[sl]
        m["xbox2d"] = xbox2d[sl]
        in_maps.append(m)
    global _LAST_EXEC_NS, _LAST_RESULTS
    import time as _time
    _t0 = _time.monotonic()
    r = run_bass_kernel_spmd(nc, in_maps, list(range(NCORES)), trace=PROFILE)
    _wall_ns = int((_time.monotonic() - _t0) * 1e9)
    _LAST_EXEC_NS = r.exec_time_ns if r.exec_time_ns is not None else _wall_ns
    _LAST_RESULTS = r
    res = r.results

    q_parts, p_parts = [], []
    hm1 = -1e30
    hm2 = -1e30
    for c in range(NCORES):
        qt = res[c]["q_t"]                      # (81, bs*9)
        au = res[c]["aux"][0]                   # (bs+8,)
        q_parts.append(qt.reshape(81, bs, 9).transpose(1, 0, 2).reshape(bs, 729))
        p_parts.append(au[:bs])
        hm1 = max(hm1, float(au[bs]))
        hm2 = max(hm2, float(au[bs + 1]))
    if not (hm1 < THRESH <= hm2):
        # Halting pattern differs from the fused 2-step fast path: recompute
        # exactly on host (slow, should never trigger for this model family).
        return _numpy_forward(inputs)
    q_values = np.concatenate(q_parts, axis=0)
    ponder = np.concatenate(p_parts, axis=0)
    return q_values, ponder


if __name__ == "__main__":
    d = np.load("/root/problem/inputs_cache.npz")
    ins = {k: d[k] for k in d.files}
    q, p = kernel(**ins)
    print("q", q.shape, "ponder", p.shape)
